# revision 2
# baseline (speedup 1.0000x reference)
"""Trainium2 Bass kernel for nn_MultiLinearCentroids (vq_codebook).

Reference math per class c (C=100, F=128, E=2048, B=512):
  one spectral-norm power-iteration step:
    sigma_c = || W_c (W_c^T u_c) || / || W_c^T u_c ||
  z = x @ W_c^T / sigma_c + b_c                         [B, F]
  probs[:, c] = exp(-||c_c - z||^2 / 2)                 [B]

Sharding: class dim padded 100 -> 104 = 8 cores x 13 classes. x replicated.
Host does only layout transforms (transpose / slice / concat / dtype cast);
all math (including sigma) runs on device.

Key design points (vs. the 127us pipeline that computed t = W^T u as a
GpSimd/DVE elementwise multiply + segmented reduce):
  - sigma via the Gram matrix: G_c = W_c W_c^T is 16 PE matmuls per class
    on the SAME wt chunks the main GEMM uses (lhsT = rhs = wt[:,k,:],
    fp32 PSUM accumulate).  Then r = G u is ONE DVE STT (in0 = G from
    PSUM, in1 = u broadcast, accum_out fp32) instead of the old
    4us GpSimd tensor_tensor + 2.3us DVE segmented reduce + 8 junk-block
    matmuls per class.  rr = r.r and ru = u.r (== ||W^T u||^2) come from
    one 2-row fp32 matmul with lhsT = r (r is written into a column slot
    adjacent to u so the rhs is a plain [128, 2] slice).
  - per-class 1/sigma chain: broadcast [rr, ru] with a ones-matmul, then
    exp(0.5 ln(ru/rr)) + one Newton step (Ln/Exp/Square all live in the
    natural_log_exp_and_others ACT table set -> single table load).
  - sq = Square(zT * invs + (b - c)) one ScalarE op -> fp16; dist2 =
    ones^T @ sq (fp16 PE partition reduce); probs row = Exp(-0.5 dist2),
    DMA'd out per class.
  - W, x ship as FP16 (PE 1 cyc/row, HBM traffic ~9.5MB); host
    pre-permutes W/x to per-partition-contiguous layouts so each DMA is
    a plain 2D copy.
  - Pipeline per iteration it: G(it) -> main GEMM(it) -> dist2(it-2) ->
    dots(it-1) -> bc(it-1) on the PE queue; r-STT(it-1) + chain(it-1) on
    DVE; Square(it-2)/probs(it-2) + chain Ln/Exp on ScalarE.  GpSimd is
    idle.  PE is the critical engine at ~5.4us/class.
"""

import numpy as np

import concourse.bass as bass
import concourse.tile as tile
from concourse import bacc


class _Bacc(bacc.Bacc):
    """Bacc whose ACT-table pass only sees natural_log_exp_and_others.

    The default pass picks the first table set containing each function
    (natural_log for Ln, exp_and_others for Exp), which alternates sets
    every class = many table loads x ~2.7us. Ln, Exp and Square all live in
    natural_log_exp_and_others, so one load covers the whole kernel."""

    def insert_act_table_loads(self):
        from concourse.hw_specs import get_activation_tables
        has_activation = any(
            isinstance(i, bacc.mybir.InstActivation)
            for b in self.main_func.blocks
            for i in b.instructions
        )
        if not has_activation:
            return
        tables = [(k, v if k == "natural_log_exp_and_others" else type(v)())
                  for k, v in get_activation_tables(self.m.arch).items()]
        bacc._bass_rust.insert_act_table_loads(self, tables)


from concourse import mybir

B = 512
C = 100
E = 2048
F = 128
NCORES = 8
CPAD = 104
CL = CPAD // NCORES  # 13 classes per core
KCH = E // 128       # 16 contraction chunks
XGRP = 4             # x chunks per staging DMA
KF = KCH * F

# misc column layout: [b.T | c.T | ones 128x128 | (r, u) column pairs]
RUC = 2 * CL + 128
MW = RUC + 2 * CL

_NC = None


def _emit(tc, d):
    nc = tc.nc
    f32 = mybir.dt.float32
    f16 = mybir.dt.float16
    mult = mybir.AluOpType.mult
    AF = mybir.ActivationFunctionType

    import contextlib
    ctx = contextlib.ExitStack()
    with ctx:
        singles = ctx.enter_context(tc.tile_pool(name="singles", bufs=1))
        wtp = ctx.enter_context(tc.tile_pool(name="wtp", bufs=8))
        sqp = ctx.enter_context(tc.tile_pool(name="sqp", bufs=2))
        smp = ctx.enter_context(tc.tile_pool(name="smp", bufs=4))
        zps = ctx.enter_context(tc.tile_pool(name="zps", bufs=3, space="PSUM"))
        gps = ctx.enter_context(tc.tile_pool(name="gps", bufs=2, space="PSUM"))
        dps = ctx.enter_context(tc.tile_pool(name="dps", bufs=1, space="PSUM"))
        dbp = ctx.enter_context(tc.tile_pool(name="dbp", bufs=2, space="PSUM"))

        # --- input staging. Two independent DGE queues: bulk W/x triggers
        # on SP, small/broadcast inputs on the ScalarE queue.
        ub = d["ubflat"]
        ubc_sb = singles.tile([128, CL * F], f32, tag="ubc")
        ub_b = bass.AP(tensor=ub.tensor, offset=ub.offset,
                       ap=[[0, 128]] + [list(a) for a in ub.ap])
        nc.scalar.dma_start(out=ubc_sb, in_=ub_b)
        misc_sb = singles.tile([128, MW], f32, tag="misc")
        nc.scalar.dma_start(out=misc_sb, in_=d["misc"][:, 0:MW])
        m16_sb = singles.tile([128, 1], f16, tag="m16")
        nc.scalar.dma_start(out=m16_sb, in_=d["m16"][:, 0:1])
        ones_sb = m16_sb[:, 0:1]
        onesrow = misc_sb[0:1, 2 * CL:2 * CL + 128]

        # W trigger groups: two singles first (fast pipeline start), then
        # pairs; issued in the prologue interleaved with x groups.
        WGROUPS = [[0], [1], [2, 3], [4, 5], [6, 7], [8, 9], [10, 11], [12]]
        wt_of = {}

        def wt_dma(gi):
            cls = WGROUPS[gi]
            wt = wtp.tile([128, len(cls), KCH, F], f16, tag="wt",
                          name=f"wtg{gi}")
            nc.sync.dma_start(
                out=wt, in_=d["wt"][:, cls[0] * KF:(cls[-1] + 1) * KF
                                    ].rearrange("p (c k f) -> p c k f",
                                                k=KCH, f=F))
            for i, c in enumerate(cls):
                wt_of[c] = (wt, i)

        def wtc(c):
            t, i = wt_of[c]
            return t[:, i, :, :]

        xt_tiles = []
        wt_dma(0)
        for g in range(KCH // XGRP):
            xg = singles.tile([128, XGRP, B], f16, tag=f"xt{g}",
                              name=f"xt{g}")
            nc.sync.dma_start(
                out=xg, in_=d["xt"][:, g * XGRP * B:(g + 1) * XGRP * B
                                    ].rearrange("p (k b) -> p k b", b=B))
            xt_tiles.append(xg)
            if 0 < g < 3:
                wt_dma(g)
        for gi in range(3, len(WGROUPS)):
            wt_dma(gi)

        negm_sb = singles.tile([F, CL], f32, tag="negm")
        nc.vector.tensor_sub(negm_sb, misc_sb[:, :CL], misc_sb[:, CL:2 * CL])

        def xchunk(k):
            return xt_tiles[k // XGRP][:, k % XGRP, :]

        st = [dict() for _ in range(CL)]

        for it in range(CL + 2):
            cb, cr, cd = it, it - 1, it - 2

            # ------ Scalar first: Square(cd) so dist2 can follow main(it)
            if 0 <= cd < CL:
                s = st[cd]
                sq = sqp.tile([F, B], f16, tag="sq")
                s["sq"] = sq
                nc.scalar.activation(
                    out=sq, in_=s["zT"], func=AF.Square,
                    bias=negm_sb[:, cd:cd + 1], scale=s["invs"])

            # ------ PE: G(cb) then main GEMM(cb), same wt chunks
            if cb < CL:
                s = st[cb]
                wt = wtc(cb)
                G = gps.tile([128, F], f32, tag="G")
                s["G"] = G
                for k in range(KCH):
                    nc.tensor.matmul(
                        G, lhsT=wt[:, k, :], rhs=wt[:, k, :],
                        start=(k == 0), stop=(k == KCH - 1))
                zT = zps.tile([F, B], f32, tag="zT")
                s["zT"] = zT
                for k in range(KCH):
                    nc.tensor.matmul(
                        zT, lhsT=wt[:, k, :], rhs=xchunk(k),
                        start=(k == 0), stop=(k == KCH - 1))

            # ------ DVE: r(cr) = G u, accumulated into the misc column
            # right before u's column so the dots rhs is a plain slice.
            if 0 <= cr < CL:
                s = st[cr]
                scr = smp.tile([128, F], f32, tag="scr")
                nc.vector.scalar_tensor_tensor(
                    out=scr, in0=s["G"], scalar=1.0,
                    in1=ubc_sb[:, cr * F:(cr + 1) * F],
                    op0=mult, op1=mult,
                    accum_out=misc_sb[:, RUC + 2 * cr:RUC + 2 * cr + 1])

            # ------ D(cd): dist2 + probs (dist2 emitted after main(it)
            # so the PE never stalls waiting on Square)
            if 0 <= cd < CL:
                s = st[cd]
                d2 = dps.tile([1, B], f32, tag="d2")
                nc.tensor.matmul(d2, lhsT=ones_sb, rhs=s["sq"],
                                 start=True, stop=True)
                probs_c = smp.tile([1, B], f32, tag="probs_c")
                nc.scalar.activation(
                    out=probs_c, in_=d2, func=AF.Exp, scale=-0.5)
                nc.sync.dma_start(out=d["out"][0:1, cd * B:(cd + 1) * B],
                                  in_=probs_c)

            # ------ dots(cr) -> broadcast -> 1/sigma chain
            if 0 <= cr < CL:
                s = st[cr]
                db = dbp.tile([128, 4], f32, tag="db")
                rcol = misc_sb[:, RUC + 2 * cr:RUC + 2 * cr + 1]
                nc.tensor.matmul(
                    db[0:1, 2:4], lhsT=rcol,
                    rhs=misc_sb[:, RUC + 2 * cr:RUC + 2 * cr + 2],
                    start=True, stop=True)
                dots_sb = smp.tile([1, 2], f32, tag="dots_sb")
                nc.scalar.activation(out=dots_sb, in_=db[0:1, 2:4],
                                     func=AF.Copy)
                nc.tensor.matmul(db[:, 0:2], lhsT=onesrow, rhs=dots_sb,
                                 start=True, stop=True)
                recip = smp.tile([128, 1], f32, tag="recip")
                nc.vector.reciprocal(recip, db[:, 0:1])
                invs2 = smp.tile([128, 1], f32, tag="invs2")
                nc.vector.tensor_mul(invs2, recip, db[:, 1:2])
                lnr = smp.tile([128, 1], f32, tag="lnr")
                nc.scalar.activation(out=lnr, in_=invs2, func=AF.Ln)
                invs0 = smp.tile([128, 1], f32, tag="invs0")
                nc.scalar.activation(out=invs0, in_=lnr, func=AF.Exp,
                                     scale=0.5)
                # one Newton step y1 = (y0 + a/y0)/2 tightens the LUT
                # exp(0.5 ln a) sqrt estimate to ~1 ulp; probs error is
                # a large multiple of the relative sigma error.
                ry = smp.tile([128, 1], f32, tag="ry")
                nc.vector.reciprocal(ry, invs0)
                ar = smp.tile([128, 1], f32, tag="ar")
                nc.vector.tensor_mul(ar, invs2, ry)
                hsum = smp.tile([128, 1], f32, tag="hsum")
                nc.vector.tensor_add(hsum, invs0, ar)
                invs = smp.tile([128, 1], f32, tag="invs")
                nc.vector.tensor_scalar_mul(invs, hsum, 0.5)
                s["invs"] = invs


def _build():
    nc = _Bacc(trn_type="TRN2", target_bir_lowering=False, debug=False,
               num_devices=NCORES)
    f32 = mybir.dt.float32
    f16 = mybir.dt.float16
    d = {
        "wt": nc.dram_tensor("wt", [128, CL * KCH * F], f16,
                             kind="ExternalInput").ap(),
        "xt": nc.dram_tensor("xt", [128, KCH * B], f16,
                             kind="ExternalInput").ap(),
        "misc": nc.dram_tensor("misc", [128, MW], f32,
                               kind="ExternalInput").ap(),
        "m16": nc.dram_tensor("m16", [128, 1], f16,
                              kind="ExternalInput").ap(),
        "ubflat": nc.dram_tensor("ubflat", [CL * F], f32,
                                 kind="ExternalInput").ap(),
        "out": nc.dram_tensor("out", [1, CL * B], f32,
                              kind="ExternalOutput").ap(),
    }
    with tile.TileContext(nc) as tc:
        _emit(tc, d)
    nc.compile()
    return nc


def _get_nc():
    global _NC
    if _NC is None:
        _NC = _build()
    return _NC


def make_in_maps(inputs):
    x = np.ascontiguousarray(inputs["x"], dtype=np.float32)
    W = np.ascontiguousarray(inputs["W"], dtype=np.float32)
    b = np.ascontiguousarray(inputs["b"], dtype=np.float32)
    u = np.ascontiguousarray(inputs["u"], dtype=np.float32)
    c = np.ascontiguousarray(inputs["c"], dtype=np.float32)
    pad = CPAD - C
    Wp = np.concatenate([W, W[:pad]], axis=0)
    bp = np.concatenate([b, b[:pad]], axis=0)
    up = np.concatenate([u, u[:pad]], axis=0)
    cp = np.concatenate([c, c[:pad]], axis=0)
    # pre-permute to per-partition-contiguous fp16 layouts so device DMAs
    # are simple 2D copies (cheap SP triggers, full-row HBM reads):
    # wt[p, c, k, f] = W[c, f, 128k+p];  xt[p, k, b] = x[b, 128k+p]
    WT = Wp.transpose(0, 2, 1).reshape(CPAD, KCH, 128, F)
    xt = np.ascontiguousarray(x.T.reshape(KCH, 128, B).transpose(1, 0, 2)
                              .reshape(128, KCH * B).astype(np.float16))
    m16 = np.ones((128, 1), dtype=np.float16)
    in_maps = []
    for ci in range(NCORES):
        sl = slice(ci * CL, (ci + 1) * CL)
        ruc = np.zeros((128, 2 * CL), dtype=np.float32)
        ruc[:, 1::2] = up[sl].T
        in_maps.append({
            "wt": np.ascontiguousarray(
                WT[sl].transpose(2, 0, 1, 3).reshape(128, CL * KCH * F)
                .astype(np.float16)),
            "xt": xt,
            "ubflat": np.ascontiguousarray(
                up[sl].reshape(-1).astype(np.float32)),
            "misc": np.ascontiguousarray(np.concatenate(
                [bp[sl].T, cp[sl].T, np.ones((128, 128), np.float32), ruc],
                axis=1)),
            "m16": m16,
        })
    return in_maps


def run_spmd(in_maps, **kw):
    from concourse.bass_utils import run_bass_kernel_spmd
    return run_bass_kernel_spmd(_get_nc(), in_maps, list(range(NCORES)), **kw)


def gather_output(results):
    rows = np.concatenate(
        [results[i]["out"].reshape(CL, B) for i in range(NCORES)], axis=0)
    return np.ascontiguousarray(rows[:C].T)  # [B, C] float32


def kernel(**inputs):
    bkr = run_spmd(make_in_maps(inputs))
    return gather_output(bkr.results)


# revision 4
# speedup vs baseline: 1.0889x; 1.0889x over previous
"""Trainium2 Bass kernel for nn_MultiLinearCentroids (vq_codebook).

Reference math per class c (C=100, F=128, E=2048, B=512):
  one spectral-norm power-iteration step:
    sigma_c = || W_c (W_c^T u_c) || / || W_c^T u_c ||
  z = x @ W_c^T / sigma_c + b_c                         [B, F]
  probs[:, c] = exp(-||c_c - z||^2 / 2)                 [B]

Sharding: class dim padded 100 -> 104 = 8 cores x 13 classes. x replicated.
Host does only layout transforms (transpose / slice / concat / dtype cast);
all math (including sigma) runs on device.

Key design points (vs. the 127us pipeline that computed t = W^T u as a
GpSimd/DVE elementwise multiply + segmented reduce):
  - sigma via the Gram matrix: G_c = W_c W_c^T is 16 PE matmuls per class
    on the SAME wt chunks the main GEMM uses (lhsT = rhs = wt[:,k,:],
    fp32 PSUM accumulate, ~57ns each).  Then r = G u is ONE DVE STT
    (in0 = G from PSUM, in1 = u broadcast, accum_out fp32 written into
    the column slot right before u's column).
  - fused dots+broadcast: one fp32 matmul with lhsT = the r column
    replicated 128x via a stride-0 free dim and rhs = the [r | u] column
    pair gives [rr, ru] = [r.r, u.r] (u.r == ||W^T u||^2) broadcast onto
    all 128 partitions in one shot -- no 1-partition dots, no ScalarE
    copy, no ones-matmul broadcast.
  - per-class 1/sigma chain on the broadcast pair: exp(0.5 ln(ru/rr)) +
    one Newton step (Ln/Exp/Square all live in the
    natural_log_exp_and_others ACT table set -> single table load).
  - sq = Square(zT * invs + (b - c)) one ScalarE op -> fp16; dist2 =
    ones^T @ sq (fp16 PE partition reduce); probs row = Exp(-0.5 dist2),
    DMA'd out per class.
  - W, x ship as FP16 (PE 1 cyc/row, HBM traffic ~9.5MB); host
    pre-permutes W/x to per-partition-contiguous layouts so each DMA is
    a plain 2D copy.  x ships in 8 half-groups interleaved with the W
    class groups so class 0 is never DMA-blocked.
  - The PE p-states ramp 0.65 -> 1.2 -> 2.4 GHz over ~3us of continuous
    work, so a handful of dependency-free warmup matmuls on an
    uninitialized SBUF tile run during the DMA prologue to finish the
    ramp before class 0's real matmuls issue.
  - Pipeline per iteration it: G(it) -> main GEMM(it) -> dist2(it-1) ->
    fused dots(it) on the PE queue; r-STT(it) + chain(it) on DVE;
    Square(it-1)/probs(it-1) + chain Ln/Exp on ScalarE.  GpSimd idle.
    PE is the critical engine at ~4.9us/class.  The last class hoists
    its dots ahead of its main GEMM to shorten the drain.
"""

import numpy as np

import concourse.bass as bass
import concourse.tile as tile
from concourse import bacc


class _Bacc(bacc.Bacc):
    """Bacc whose ACT-table pass only sees natural_log_exp_and_others.

    The default pass picks the first table set containing each function
    (natural_log for Ln, exp_and_others for Exp), which alternates sets
    every class = many table loads x ~2.7us. Ln, Exp and Square all live in
    natural_log_exp_and_others, so one load covers the whole kernel."""

    def insert_act_table_loads(self):
        from concourse.hw_specs import get_activation_tables
        has_activation = any(
            isinstance(i, bacc.mybir.InstActivation)
            for b in self.main_func.blocks
            for i in b.instructions
        )
        if not has_activation:
            return
        tables = [(k, v if k == "natural_log_exp_and_others" else type(v)())
                  for k, v in get_activation_tables(self.m.arch).items()]
        bacc._bass_rust.insert_act_table_loads(self, tables)


from concourse import mybir

B = 512
C = 100
E = 2048
F = 128
NCORES = 8
CPAD = 104
CL = CPAD // NCORES  # 13 classes per core
KCH = E // 128       # 16 contraction chunks
KF = KCH * F
NWARM = 11           # p-state warmup matmuls during the DMA prologue

# misc column layout: [b.T | c.T | (r, u) column pairs]
RUC = 2 * CL
MW = 4 * CL

_NC = None


def _emit(tc, d):
    nc = tc.nc
    f32 = mybir.dt.float32
    f16 = mybir.dt.float16
    mult = mybir.AluOpType.mult
    AF = mybir.ActivationFunctionType

    import contextlib
    ctx = contextlib.ExitStack()
    with ctx:
        singles = ctx.enter_context(tc.tile_pool(name="singles", bufs=1))
        wtp = ctx.enter_context(tc.tile_pool(name="wtp", bufs=8))
        sqp = ctx.enter_context(tc.tile_pool(name="sqp", bufs=2))
        smp = ctx.enter_context(tc.tile_pool(name="smp", bufs=4))
        zps = ctx.enter_context(tc.tile_pool(name="zps", bufs=2, space="PSUM"))
        gps = ctx.enter_context(tc.tile_pool(name="gps", bufs=2, space="PSUM"))
        dps = ctx.enter_context(tc.tile_pool(name="dps", bufs=1, space="PSUM"))
        dbp = ctx.enter_context(tc.tile_pool(name="dbp", bufs=2, space="PSUM"))
        wup = ctx.enter_context(tc.tile_pool(name="wup", bufs=1, space="PSUM"))

        # --- PE p-state warmup: dependency-free matmuls on an
        # uninitialized SBUF tile; results land in a dedicated junk PSUM
        # bank that is never read.
        warm_sb = singles.tile([128, B], f16, tag="warm")
        nc.gpsimd.memset(warm_sb, 0.0)
        warm_ps = wup.tile([128, B], f32, tag="warmp")
        for _ in range(NWARM):
            nc.tensor.matmul(warm_ps, lhsT=warm_sb[:, 0:128], rhs=warm_sb,
                             start=True, stop=True)

        # --- input staging. Two independent DGE queues: bulk W/x triggers
        # on SP, small/broadcast inputs on the ScalarE queue.
        ub = d["ubflat"]
        ubc_sb = singles.tile([128, CL * F], f32, tag="ubc")
        ub_b = bass.AP(tensor=ub.tensor, offset=ub.offset,
                       ap=[[0, 128]] + [list(a) for a in ub.ap])
        nc.scalar.dma_start(out=ubc_sb, in_=ub_b)
        misc_sb = singles.tile([128, MW], f32, tag="misc")
        nc.scalar.dma_start(out=misc_sb, in_=d["misc"][:, 0:MW])
        m16_sb = singles.tile([128, 1], f16, tag="m16")
        nc.scalar.dma_start(out=m16_sb, in_=d["m16"][:, 0:1])
        ones_sb = m16_sb[:, 0:1]

        # W trigger groups: two singles first (fast pipeline start), then
        # pairs; issued interleaved with half-size x groups so class 0's
        # main GEMM is never waiting on x.
        WGROUPS = [[0], [1], [2, 3], [4, 5], [6, 7], [8, 9], [10, 11], [12]]
        wt_of = {}

        def wt_dma(gi):
            cls = WGROUPS[gi]
            wt = wtp.tile([128, len(cls), KCH, F], f16, tag="wt",
                          name=f"wtg{gi}")
            nc.sync.dma_start(
                out=wt, in_=d["wt"][:, cls[0] * KF:(cls[-1] + 1) * KF
                                    ].rearrange("p (c k f) -> p c k f",
                                                k=KCH, f=F))
            for i, c in enumerate(cls):
                wt_of[c] = (wt, i)

        def wtc(c):
            t, i = wt_of[c]
            return t[:, i, :, :]

        XG = 2  # x chunks per staging DMA
        xt_tiles = []
        wt_dma(0)
        for g in range(KCH // XG):
            xg = singles.tile([128, XG, B], f16, tag=f"xt{g}",
                              name=f"xt{g}")
            nc.sync.dma_start(
                out=xg, in_=d["xt"][:, g * XG * B:(g + 1) * XG * B
                                    ].rearrange("p (k b) -> p k b", b=B))
            xt_tiles.append(xg)
            if 0 < g < 3:
                wt_dma(g)
        for gi in range(3, len(WGROUPS)):
            wt_dma(gi)

        negm_sb = singles.tile([F, CL], f32, tag="negm")
        nc.vector.tensor_sub(negm_sb, misc_sb[:, :CL], misc_sb[:, CL:2 * CL])

        def xchunk(k):
            return xt_tiles[k // XG][:, k % XG, :]

        st = [dict() for _ in range(CL)]

        def emit_sigma(cr):
            """r = G u (DVE STT), fused broadcast dots (PE), 1/sigma chain."""
            s = st[cr]
            scr = smp.tile([128, F], f32, tag="scr")
            rcol = misc_sb[:, RUC + 2 * cr:RUC + 2 * cr + 1]
            nc.vector.scalar_tensor_tensor(
                out=scr, in0=s["G"], scalar=1.0,
                in1=ubc_sb[:, cr * F:(cr + 1) * F],
                op0=mult, op1=mult, accum_out=rcol)
            # fused dots+broadcast: lhsT = r replicated 128x (stride-0
            # free dim), rhs = [r | u] -> out[m, :] = [r.r, u.r] for all m
            db = dbp.tile([128, 2], f32, tag="db")
            s["db"] = db
            rrep = bass.AP(tensor=rcol.tensor, offset=rcol.offset,
                           ap=[list(rcol.ap[0]), [0, 128]])
            nc.tensor.matmul(
                db, lhsT=rrep,
                rhs=misc_sb[:, RUC + 2 * cr:RUC + 2 * cr + 2],
                start=True, stop=True)

        def emit_chain(cr):
            s = st[cr]
            db = s["db"]
            recip = smp.tile([128, 1], f32, tag="recip")
            nc.vector.reciprocal(recip, db[:, 0:1])
            invs2 = smp.tile([128, 1], f32, tag="invs2")
            nc.vector.tensor_mul(invs2, recip, db[:, 1:2])
            lnr = smp.tile([128, 1], f32, tag="lnr")
            nc.scalar.activation(out=lnr, in_=invs2, func=AF.Ln)
            invs0 = smp.tile([128, 1], f32, tag="invs0")
            nc.scalar.activation(out=invs0, in_=lnr, func=AF.Exp, scale=0.5)
            # one Newton step y1 = (y0 + a/y0)/2 tightens the LUT
            # exp(0.5 ln a) sqrt estimate to ~1 ulp; probs error is
            # a large multiple of the relative sigma error.
            ry = smp.tile([128, 1], f32, tag="ry")
            nc.vector.reciprocal(ry, invs0)
            ar = smp.tile([128, 1], f32, tag="ar")
            nc.vector.tensor_mul(ar, invs2, ry)
            hsum = smp.tile([128, 1], f32, tag="hsum")
            nc.vector.tensor_add(hsum, invs0, ar)
            invs = smp.tile([128, 1], f32, tag="invs")
            nc.vector.tensor_scalar_mul(invs, hsum, 0.5)
            s["invs"] = invs

        for it in range(CL + 1):
            cb, cd = it, it - 1
            last = cb == CL - 1

            # ------ Scalar first: Square(cd) so dist2 can follow main(it)
            if 0 <= cd < CL:
                s = st[cd]
                sq = sqp.tile([F, B], f16, tag="sq")
                s["sq"] = sq
                nc.scalar.activation(
                    out=sq, in_=s["zT"], func=AF.Square,
                    bias=negm_sb[:, cd:cd + 1], scale=s["invs"])

            # ------ PE: G(cb) then main GEMM(cb), same wt chunks
            if cb < CL:
                s = st[cb]
                wt = wtc(cb)
                G = gps.tile([128, F], f32, tag="G")
                s["G"] = G
                for k in range(KCH):
                    nc.tensor.matmul(
                        G, lhsT=wt[:, k, :], rhs=wt[:, k, :],
                        start=(k == 0), stop=(k == KCH - 1))
                # last class: sigma path ahead of the main GEMM so its
                # chain finishes during the GEMM instead of in the drain
                if last:
                    emit_sigma(cb)
                zT = zps.tile([F, B], f32, tag="zT")
                s["zT"] = zT
                for k in range(KCH):
                    nc.tensor.matmul(
                        zT, lhsT=wt[:, k, :], rhs=xchunk(k),
                        start=(k == 0), stop=(k == KCH - 1))

            # ------ D(cd): dist2 + probs (dist2 emitted after main(it)
            # so the PE never stalls waiting on Square)
            if 0 <= cd < CL:
                s = st[cd]
                d2 = dps.tile([1, B], f32, tag="d2")
                nc.tensor.matmul(d2, lhsT=ones_sb, rhs=s["sq"],
                                 start=True, stop=True)
                probs_c = smp.tile([1, B], f32, tag="probs_c")
                nc.scalar.activation(
                    out=probs_c, in_=d2, func=AF.Exp, scale=-0.5)
                nc.sync.dma_start(out=d["out"][0:1, cd * B:(cd + 1) * B],
                                  in_=probs_c)

            # ------ sigma(cb): r, dots, chain
            if cb < CL:
                if not last:
                    emit_sigma(cb)
                emit_chain(cb)


def _build():
    nc = _Bacc(trn_type="TRN2", target_bir_lowering=False, debug=False,
               num_devices=NCORES)
    f32 = mybir.dt.float32
    f16 = mybir.dt.float16
    d = {
        "wt": nc.dram_tensor("wt", [128, CL * KCH * F], f16,
                             kind="ExternalInput").ap(),
        "xt": nc.dram_tensor("xt", [128, KCH * B], f16,
                             kind="ExternalInput").ap(),
        "misc": nc.dram_tensor("misc", [128, MW], f32,
                               kind="ExternalInput").ap(),
        "m16": nc.dram_tensor("m16", [128, 1], f16,
                              kind="ExternalInput").ap(),
        "ubflat": nc.dram_tensor("ubflat", [CL * F], f32,
                                 kind="ExternalInput").ap(),
        "out": nc.dram_tensor("out", [1, CL * B], f32,
                              kind="ExternalOutput").ap(),
    }
    with tile.TileContext(nc) as tc:
        _emit(tc, d)
    nc.compile()
    return nc


def _get_nc():
    global _NC
    if _NC is None:
        _NC = _build()
    return _NC


def make_in_maps(inputs):
    x = np.ascontiguousarray(inputs["x"], dtype=np.float32)
    W = np.ascontiguousarray(inputs["W"], dtype=np.float32)
    b = np.ascontiguousarray(inputs["b"], dtype=np.float32)
    u = np.ascontiguousarray(inputs["u"], dtype=np.float32)
    c = np.ascontiguousarray(inputs["c"], dtype=np.float32)
    pad = CPAD - C
    Wp = np.concatenate([W, W[:pad]], axis=0)
    bp = np.concatenate([b, b[:pad]], axis=0)
    up = np.concatenate([u, u[:pad]], axis=0)
    cp = np.concatenate([c, c[:pad]], axis=0)
    # pre-permute to per-partition-contiguous fp16 layouts so device DMAs
    # are simple 2D copies (cheap SP triggers, full-row HBM reads):
    # wt[p, c, k, f] = W[c, f, 128k+p];  xt[p, k, b] = x[b, 128k+p]
    WT = Wp.transpose(0, 2, 1).reshape(CPAD, KCH, 128, F)
    xt = np.ascontiguousarray(x.T.reshape(KCH, 128, B).transpose(1, 0, 2)
                              .reshape(128, KCH * B).astype(np.float16))
    m16 = np.ones((128, 1), dtype=np.float16)
    in_maps = []
    for ci in range(NCORES):
        sl = slice(ci * CL, (ci + 1) * CL)
        ruc = np.zeros((128, 2 * CL), dtype=np.float32)
        ruc[:, 1::2] = up[sl].T
        in_maps.append({
            "wt": np.ascontiguousarray(
                WT[sl].transpose(2, 0, 1, 3).reshape(128, CL * KCH * F)
                .astype(np.float16)),
            "xt": xt,
            "ubflat": np.ascontiguousarray(
                up[sl].reshape(-1).astype(np.float32)),
            "misc": np.ascontiguousarray(np.concatenate(
                [bp[sl].T, cp[sl].T, ruc], axis=1)),
            "m16": m16,
        })
    return in_maps


def run_spmd(in_maps, **kw):
    from concourse.bass_utils import run_bass_kernel_spmd
    return run_bass_kernel_spmd(_get_nc(), in_maps, list(range(NCORES)), **kw)


def gather_output(results):
    rows = np.concatenate(
        [results[i]["out"].reshape(CL, B) for i in range(NCORES)], axis=0)
    return np.ascontiguousarray(rows[:C].T)  # [B, C] float32


def kernel(**inputs):
    bkr = run_spmd(make_in_maps(inputs))
    return gather_output(bkr.results)


# revision 6
# speedup vs baseline: 1.1095x; 1.0189x over previous
"""Trainium2 Bass kernel for nn_MultiLinearCentroids (vq_codebook).

Reference math per class c (C=100, F=128, E=2048, B=512):
  one spectral-norm power-iteration step:
    sigma_c = || W_c (W_c^T u_c) || / || W_c^T u_c ||
  z = x @ W_c^T / sigma_c + b_c                         [B, F]
  probs[:, c] = exp(-||c_c - z||^2 / 2)                 [B]

Sharding: class dim padded 100 -> 104 = 8 cores x 13 classes. x replicated.
Host does only layout transforms (transpose / slice / concat / dtype cast);
all math (including sigma) runs on device.

Key design points (vs. the 127us pipeline that computed t = W^T u as a
GpSimd/DVE elementwise multiply + segmented reduce):
  - sigma via the Gram matrix: G_c = W_c W_c^T is 16 PE matmuls per class
    on the SAME wt chunks the main GEMM uses (lhsT = rhs = wt[:,k,:],
    fp32 PSUM accumulate, ~57ns each).  Then r = G u is ONE DVE STT
    (in0 = G from PSUM, in1 = u broadcast, accum_out fp32 written into
    the column slot right before u's column).
  - fused dots+broadcast: one fp32 matmul with lhsT = the r column
    replicated 128x via a stride-0 free dim and rhs = the [r | u] column
    pair gives [rr, ru] = [r.r, u.r] (u.r == ||W^T u||^2) broadcast onto
    all 128 partitions in one shot -- no 1-partition dots, no ScalarE
    copy, no ones-matmul broadcast.
  - per-class 1/sigma chain on the broadcast pair: exp(0.5 ln(ru/rr)) +
    one Newton step (Ln/Exp/Square all live in the
    natural_log_exp_and_others ACT table set -> single table load).
  - sq = Square(zT * invs + (b - c)) one ScalarE op -> fp16; dist2 =
    ones^T @ sq (fp16 PE partition reduce); probs row = Exp(-0.5 dist2),
    DMA'd out per class.
  - W, x ship as FP16 (PE 1 cyc/row, HBM traffic ~9.5MB); host
    pre-permutes W/x to per-partition-contiguous layouts so each DMA is
    a plain 2D copy.  x ships in 8 half-groups interleaved with the W
    class groups so class 0 is never DMA-blocked.
  - The PE p-states ramp 0.65 -> 1.2 -> 2.4 GHz over ~3us of continuous
    work, so a handful of dependency-free warmup matmuls on an
    uninitialized SBUF tile run during the DMA prologue to finish the
    ramp before class 0's real matmuls issue.
  - Pipeline per iteration it: G(it) -> main GEMM(it) -> dist2(it-1) ->
    fused dots(it) on the PE queue; r-STT(it) + chain(it) on DVE;
    Square(it-1)/probs(it-1) + chain Ln/Exp on ScalarE.  GpSimd idle.
    PE is the critical engine at ~4.9us/class.  The last class hoists
    its dots ahead of its main GEMM to shorten the drain.
"""

import numpy as np

import concourse.bass as bass
import concourse.tile as tile
from concourse import bacc


class _Bacc(bacc.Bacc):
    """Bacc whose ACT-table pass only sees natural_log_exp_and_others.

    The default pass picks the first table set containing each function
    (natural_log for Ln, exp_and_others for Exp), which alternates sets
    every class = many table loads x ~2.7us. Ln, Exp and Square all live in
    natural_log_exp_and_others, so one load covers the whole kernel."""

    def insert_act_table_loads(self):
        from concourse.hw_specs import get_activation_tables
        has_activation = any(
            isinstance(i, bacc.mybir.InstActivation)
            for b in self.main_func.blocks
            for i in b.instructions
        )
        if not has_activation:
            return
        tables = [(k, v if k == "natural_log_exp_and_others" else type(v)())
                  for k, v in get_activation_tables(self.m.arch).items()]
        bacc._bass_rust.insert_act_table_loads(self, tables)


from concourse import mybir

B = 512
C = 100
E = 2048
F = 128
NCORES = 8
CPAD = 104
CL = CPAD // NCORES  # 13 classes per core
KCH = E // 128       # 16 contraction chunks
KF = KCH * F
NWARM = 5            # p-state warmup matmuls during the DMA prologue

# misc column layout: [b.T | c.T | (r, u) column pairs]
RUC = 2 * CL
MW = 4 * CL

_NC = None


def _emit(tc, d):
    nc = tc.nc
    f32 = mybir.dt.float32
    f16 = mybir.dt.float16
    mult = mybir.AluOpType.mult
    AF = mybir.ActivationFunctionType

    import contextlib
    ctx = contextlib.ExitStack()
    with ctx:
        singles = ctx.enter_context(tc.tile_pool(name="singles", bufs=1))
        wtp = ctx.enter_context(tc.tile_pool(name="wtp", bufs=8))
        sqp = ctx.enter_context(tc.tile_pool(name="sqp", bufs=2))
        smp = ctx.enter_context(tc.tile_pool(name="smp", bufs=4))
        zps = ctx.enter_context(tc.tile_pool(name="zps", bufs=2, space="PSUM"))
        gps = ctx.enter_context(tc.tile_pool(name="gps", bufs=2, space="PSUM"))
        dps = ctx.enter_context(tc.tile_pool(name="dps", bufs=1, space="PSUM"))
        dbp = ctx.enter_context(tc.tile_pool(name="dbp", bufs=2, space="PSUM"))
        wup = ctx.enter_context(tc.tile_pool(name="wup", bufs=1, space="PSUM"))

        # --- PE p-state warmup: dependency-free matmuls on an
        # uninitialized SBUF tile; results land in a dedicated junk PSUM
        # bank that is never read.
        warm_sb = singles.tile([128, B], f16, tag="warm")
        nc.gpsimd.memset(warm_sb, 0.0)
        warm_ps = wup.tile([128, B], f32, tag="warmp")
        for _ in range(NWARM):
            nc.tensor.matmul(warm_ps, lhsT=warm_sb[:, 0:128], rhs=warm_sb,
                             start=True, stop=True)

        # --- input staging. Two independent DGE queues: bulk W/x triggers
        # on SP, small/broadcast inputs on the ScalarE queue.
        ub = d["ubflat"]
        ubc_sb = singles.tile([128, CL * F], f32, tag="ubc")
        ub_b = bass.AP(tensor=ub.tensor, offset=ub.offset,
                       ap=[[0, 128]] + [list(a) for a in ub.ap])
        nc.scalar.dma_start(out=ubc_sb, in_=ub_b)
        misc_sb = singles.tile([128, MW], f32, tag="misc")
        nc.scalar.dma_start(out=misc_sb, in_=d["misc"][:, 0:MW])
        m16_sb = singles.tile([128, 1], f16, tag="m16")
        nc.scalar.dma_start(out=m16_sb, in_=d["m16"][:, 0:1])
        ones_sb = m16_sb[:, 0:1]

        # W trigger groups: two singles first (fast pipeline start), then
        # pairs; issued interleaved with half-size x groups so class 0's
        # main GEMM is never waiting on x.
        WGROUPS = [[0], [1], [2, 3], [4, 5], [6, 7], [8, 9], [10, 11], [12]]
        wt_of = {}

        def wt_dma(gi):
            cls = WGROUPS[gi]
            wt = wtp.tile([128, len(cls), KCH, F], f16, tag="wt",
                          name=f"wtg{gi}")
            nc.sync.dma_start(
                out=wt, in_=d["wt"][:, cls[0] * KF:(cls[-1] + 1) * KF
                                    ].rearrange("p (c k f) -> p c k f",
                                                k=KCH, f=F))
            for i, c in enumerate(cls):
                wt_of[c] = (wt, i)

        def wtc(c):
            t, i = wt_of[c]
            return t[:, i, :, :]

        # wt0 first (class 0's G gates on it), then ALL of x (class 0's
        # main GEMM consumes every chunk before class 1 even starts),
        # then the remaining W groups which trail the compute easily.
        XG = 2  # x chunks per staging DMA
        xt_tiles = []
        wt_dma(0)
        for g in range(KCH // XG):
            xg = singles.tile([128, XG, B], f16, tag=f"xt{g}",
                              name=f"xt{g}")
            nc.sync.dma_start(
                out=xg, in_=d["xt"][:, g * XG * B:(g + 1) * XG * B
                                    ].rearrange("p (k b) -> p k b", b=B))
            xt_tiles.append(xg)
        for gi in range(1, len(WGROUPS)):
            wt_dma(gi)

        negm_sb = singles.tile([F, CL], f32, tag="negm")
        nc.vector.tensor_sub(negm_sb, misc_sb[:, :CL], misc_sb[:, CL:2 * CL])

        def xchunk(k):
            return xt_tiles[k // XG][:, k % XG, :]

        st = [dict() for _ in range(CL)]

        def emit_sigma(cr):
            """r = G u (DVE STT), fused broadcast dots (PE), 1/sigma chain."""
            s = st[cr]
            scr = smp.tile([128, F], f32, tag="scr")
            rcol = misc_sb[:, RUC + 2 * cr:RUC + 2 * cr + 1]
            nc.vector.scalar_tensor_tensor(
                out=scr, in0=s["G"], scalar=1.0,
                in1=ubc_sb[:, cr * F:(cr + 1) * F],
                op0=mult, op1=mult, accum_out=rcol)
            # fused dots+broadcast: lhsT = r replicated 128x (stride-0
            # free dim), rhs = [r | u] -> out[m, :] = [r.r, u.r] for all m
            db = dbp.tile([128, 2], f32, tag="db")
            s["db"] = db
            rrep = bass.AP(tensor=rcol.tensor, offset=rcol.offset,
                           ap=[list(rcol.ap[0]), [0, 128]])
            nc.tensor.matmul(
                db, lhsT=rrep,
                rhs=misc_sb[:, RUC + 2 * cr:RUC + 2 * cr + 2],
                start=True, stop=True)

        def emit_chain(cr):
            s = st[cr]
            db = s["db"]
            recip = smp.tile([128, 1], f32, tag="recip")
            nc.vector.reciprocal(recip, db[:, 0:1])
            invs2 = smp.tile([128, 1], f32, tag="invs2")
            nc.vector.tensor_mul(invs2, recip, db[:, 1:2])
            lnr = smp.tile([128, 1], f32, tag="lnr")
            nc.scalar.activation(out=lnr, in_=invs2, func=AF.Ln)
            invs0 = smp.tile([128, 1], f32, tag="invs0")
            nc.scalar.activation(out=invs0, in_=lnr, func=AF.Exp, scale=0.5)
            # one Newton step y1 = (y0 + a/y0)/2 tightens the LUT
            # exp(0.5 ln a) sqrt estimate to ~1 ulp; probs error is
            # a large multiple of the relative sigma error.
            ry = smp.tile([128, 1], f32, tag="ry")
            nc.vector.reciprocal(ry, invs0)
            ar = smp.tile([128, 1], f32, tag="ar")
            nc.vector.tensor_mul(ar, invs2, ry)
            hsum = smp.tile([128, 1], f32, tag="hsum")
            nc.vector.tensor_add(hsum, invs0, ar)
            invs = smp.tile([128, 1], f32, tag="invs")
            nc.vector.tensor_scalar_mul(invs, hsum, 0.5)
            s["invs"] = invs

        for it in range(CL + 1):
            cb, cd = it, it - 1
            last = cb == CL - 1

            # ------ Scalar first: Square(cd) so dist2 can follow main(it)
            if 0 <= cd < CL:
                s = st[cd]
                sq = sqp.tile([F, B], f16, tag="sq")
                s["sq"] = sq
                nc.scalar.activation(
                    out=sq, in_=s["zT"], func=AF.Square,
                    bias=negm_sb[:, cd:cd + 1], scale=s["invs"])

            # ------ PE: G(cb) then main GEMM(cb), same wt chunks
            if cb < CL:
                s = st[cb]
                wt = wtc(cb)
                G = gps.tile([128, F], f32, tag="G")
                s["G"] = G
                for k in range(KCH):
                    nc.tensor.matmul(
                        G, lhsT=wt[:, k, :], rhs=wt[:, k, :],
                        start=(k == 0), stop=(k == KCH - 1))
                # last class: sigma path ahead of the main GEMM so its
                # chain finishes during the GEMM instead of in the drain
                if last:
                    emit_sigma(cb)
                zT = zps.tile([F, B], f32, tag="zT")
                s["zT"] = zT
                for k in range(KCH):
                    nc.tensor.matmul(
                        zT, lhsT=wt[:, k, :], rhs=xchunk(k),
                        start=(k == 0), stop=(k == KCH - 1))

            # ------ D(cd): dist2 + probs (dist2 emitted after main(it)
            # so the PE never stalls waiting on Square)
            if 0 <= cd < CL:
                s = st[cd]
                d2 = dps.tile([1, B], f32, tag="d2")
                nc.tensor.matmul(d2, lhsT=ones_sb, rhs=s["sq"],
                                 start=True, stop=True)
                probs_c = smp.tile([1, B], f32, tag="probs_c")
                nc.scalar.activation(
                    out=probs_c, in_=d2, func=AF.Exp, scale=-0.5)
                nc.sync.dma_start(out=d["out"][0:1, cd * B:(cd + 1) * B],
                                  in_=probs_c)

            # ------ sigma(cb): r, dots, chain
            if cb < CL:
                if not last:
                    emit_sigma(cb)
                emit_chain(cb)


def _build():
    nc = _Bacc(trn_type="TRN2", target_bir_lowering=False, debug=False,
               num_devices=NCORES)
    f32 = mybir.dt.float32
    f16 = mybir.dt.float16
    d = {
        "wt": nc.dram_tensor("wt", [128, CL * KCH * F], f16,
                             kind="ExternalInput").ap(),
        "xt": nc.dram_tensor("xt", [128, KCH * B], f16,
                             kind="ExternalInput").ap(),
        "misc": nc.dram_tensor("misc", [128, MW], f32,
                               kind="ExternalInput").ap(),
        "m16": nc.dram_tensor("m16", [128, 1], f16,
                              kind="ExternalInput").ap(),
        "ubflat": nc.dram_tensor("ubflat", [CL * F], f32,
                                 kind="ExternalInput").ap(),
        "out": nc.dram_tensor("out", [1, CL * B], f32,
                              kind="ExternalOutput").ap(),
    }
    with tile.TileContext(nc) as tc:
        _emit(tc, d)
    nc.compile()
    return nc


def _get_nc():
    global _NC
    if _NC is None:
        _NC = _build()
    return _NC


def make_in_maps(inputs):
    x = np.ascontiguousarray(inputs["x"], dtype=np.float32)
    W = np.ascontiguousarray(inputs["W"], dtype=np.float32)
    b = np.ascontiguousarray(inputs["b"], dtype=np.float32)
    u = np.ascontiguousarray(inputs["u"], dtype=np.float32)
    c = np.ascontiguousarray(inputs["c"], dtype=np.float32)
    pad = CPAD - C
    Wp = np.concatenate([W, W[:pad]], axis=0)
    bp = np.concatenate([b, b[:pad]], axis=0)
    up = np.concatenate([u, u[:pad]], axis=0)
    cp = np.concatenate([c, c[:pad]], axis=0)
    # pre-permute to per-partition-contiguous fp16 layouts so device DMAs
    # are simple 2D copies (cheap SP triggers, full-row HBM reads):
    # wt[p, c, k, f] = W[c, f, 128k+p];  xt[p, k, b] = x[b, 128k+p]
    WT = Wp.transpose(0, 2, 1).reshape(CPAD, KCH, 128, F)
    xt = np.ascontiguousarray(x.T.reshape(KCH, 128, B).transpose(1, 0, 2)
                              .reshape(128, KCH * B).astype(np.float16))
    m16 = np.ones((128, 1), dtype=np.float16)
    in_maps = []
    for ci in range(NCORES):
        sl = slice(ci * CL, (ci + 1) * CL)
        ruc = np.zeros((128, 2 * CL), dtype=np.float32)
        ruc[:, 1::2] = up[sl].T
        in_maps.append({
            "wt": np.ascontiguousarray(
                WT[sl].transpose(2, 0, 1, 3).reshape(128, CL * KCH * F)
                .astype(np.float16)),
            "xt": xt,
            "ubflat": np.ascontiguousarray(
                up[sl].reshape(-1).astype(np.float32)),
            "misc": np.ascontiguousarray(np.concatenate(
                [bp[sl].T, cp[sl].T, ruc], axis=1)),
            "m16": m16,
        })
    return in_maps


def run_spmd(in_maps, **kw):
    from concourse.bass_utils import run_bass_kernel_spmd
    return run_bass_kernel_spmd(_get_nc(), in_maps, list(range(NCORES)), **kw)


def gather_output(results):
    rows = np.concatenate(
        [results[i]["out"].reshape(CL, B) for i in range(NCORES)], axis=0)
    return np.ascontiguousarray(rows[:C].T)  # [B, C] float32


def kernel(**inputs):
    bkr = run_spmd(make_in_maps(inputs))
    return gather_output(bkr.results)


# revision 12
# speedup vs baseline: 1.1118x; 1.0020x over previous
"""Trainium2 Bass kernel for nn_MultiLinearCentroids (vq_codebook).

Reference math per class c (C=100, F=128, E=2048, B=512):
  one spectral-norm power-iteration step:
    sigma_c = || W_c (W_c^T u_c) || / || W_c^T u_c ||
  z = x @ W_c^T / sigma_c + b_c                         [B, F]
  probs[:, c] = exp(-||c_c - z||^2 / 2)                 [B]

Sharding: class dim padded 100 -> 104 = 8 cores x 13 classes. x replicated.
Host does only layout transforms (transpose / slice / concat / dtype cast);
all math (including sigma) runs on device.

Key design points (vs. the 127us pipeline that computed t = W^T u as a
GpSimd/DVE elementwise multiply + segmented reduce):
  - sigma via the Gram matrix: G_c = W_c W_c^T is 16 PE matmuls per class
    on the SAME wt chunks the main GEMM uses (lhsT = rhs = wt[:,k,:],
    fp32 PSUM accumulate, ~57ns each).  Then r = G u is ONE DVE STT
    (in0 = G from PSUM, in1 = u broadcast, accum_out fp32 written into
    the column slot right before u's column).
  - fused dots+broadcast: one fp32 matmul with lhsT = the r column
    replicated 128x via a stride-0 free dim and rhs = the [r | u] column
    pair gives [rr, ru] = [r.r, u.r] (u.r == ||W^T u||^2) broadcast onto
    all 128 partitions in one shot -- no 1-partition dots, no ScalarE
    copy, no ones-matmul broadcast.
  - per-class 1/sigma chain on the broadcast pair: exp(0.5 ln(ru/rr)) +
    one Newton step (Ln/Exp/Square all live in the
    natural_log_exp_and_others ACT table set -> single table load).
  - sq = Square(zT * invs + (b - c)) one ScalarE op -> fp16; dist2 =
    ones^T @ sq (fp16 PE partition reduce); probs row = Exp(-0.5 dist2),
    DMA'd out per class.
  - W, x ship as FP16 (PE 1 cyc/row, HBM traffic ~9.5MB); host
    pre-permutes W/x to per-partition-contiguous layouts so each DMA is
    a plain 2D copy.  x ships in 8 half-groups interleaved with the W
    class groups so class 0 is never DMA-blocked.
  - The PE p-states ramp 0.65 -> 1.2 -> 2.4 GHz over ~3us of continuous
    work, so a handful of dependency-free warmup matmuls on an
    uninitialized SBUF tile run during the DMA prologue to finish the
    ramp before class 0's real matmuls issue.
  - Pipeline per iteration it: G(it) -> main GEMM(it) -> dist2(it-1) ->
    fused dots(it) on the PE queue; r-STT(it) + chain(it) on DVE;
    Square(it-1)/probs(it-1) + chain Ln/Exp on ScalarE.  GpSimd idle.
    PE is the critical engine at ~4.9us/class.  The last class hoists
    its dots ahead of its main GEMM to shorten the drain.
"""

import numpy as np

import concourse.bass as bass
import concourse.tile as tile
from concourse import bacc


class _Bacc(bacc.Bacc):
    """Bacc whose ACT-table pass only sees natural_log_exp_and_others.

    The default pass picks the first table set containing each function
    (natural_log for Ln, exp_and_others for Exp), which alternates sets
    every class = many table loads x ~2.7us. Ln, Exp and Square all live in
    natural_log_exp_and_others, so one load covers the whole kernel."""

    def insert_act_table_loads(self):
        from concourse.hw_specs import get_activation_tables
        has_activation = any(
            isinstance(i, bacc.mybir.InstActivation)
            for b in self.main_func.blocks
            for i in b.instructions
        )
        if not has_activation:
            return
        tables = [(k, v if k == "natural_log_exp_and_others" else type(v)())
                  for k, v in get_activation_tables(self.m.arch).items()]
        bacc._bass_rust.insert_act_table_loads(self, tables)


from concourse import mybir

B = 512
C = 100
E = 2048
F = 128
NCORES = 8
CPAD = 104
CL = CPAD // NCORES  # 13 classes per core
KCH = E // 128       # 16 contraction chunks
KF = KCH * F
NWARM = 5            # p-state warmup matmuls during the DMA prologue

# misc column layout: [b.T | c.T | (r, u) column pairs]
RUC = 2 * CL
MW = 4 * CL

_NC = None


def _emit(tc, d):
    nc = tc.nc
    f32 = mybir.dt.float32
    f16 = mybir.dt.float16
    mult = mybir.AluOpType.mult
    AF = mybir.ActivationFunctionType

    import contextlib
    ctx = contextlib.ExitStack()
    with ctx:
        singles = ctx.enter_context(tc.tile_pool(name="singles", bufs=1))
        wtp = ctx.enter_context(tc.tile_pool(name="wtp", bufs=8))
        sqp = ctx.enter_context(tc.tile_pool(name="sqp", bufs=2))
        smp = ctx.enter_context(tc.tile_pool(name="smp", bufs=4))
        zps = ctx.enter_context(tc.tile_pool(name="zps", bufs=2, space="PSUM"))
        gps = ctx.enter_context(tc.tile_pool(name="gps", bufs=2, space="PSUM"))
        dps = ctx.enter_context(tc.tile_pool(name="dps", bufs=1, space="PSUM"))
        dbp = ctx.enter_context(tc.tile_pool(name="dbp", bufs=2, space="PSUM"))
        wup = ctx.enter_context(tc.tile_pool(name="wup", bufs=1, space="PSUM"))

        # --- PE p-state warmup: dependency-free matmuls on an
        # uninitialized SBUF tile; results land in a dedicated junk PSUM
        # bank that is never read.
        warm_sb = singles.tile([128, B], f16, tag="warm")
        nc.gpsimd.memset(warm_sb, 0.0)
        warm_ps = wup.tile([128, B], f32, tag="warmp")
        for _ in range(NWARM):
            nc.tensor.matmul(warm_ps, lhsT=warm_sb[:, 0:128], rhs=warm_sb,
                             start=True, stop=True)

        # --- input staging across the two hardware DGE queues (SP +
        # ScalarE): the DMA engines ramp like the PE (~170 GB/s early),
        # so the critical prologue bytes (wt0 + x + u for early classes)
        # are split so both queues land them in parallel.
        ub = d["ubflat"]
        ubc_sb = singles.tile([128, CL * F], f16, tag="ubc")
        misc_sb = singles.tile([128, MW], f32, tag="misc")
        m16_sb = singles.tile([128, 1], f16, tag="m16")
        ones_sb = m16_sb[:, 0:1]

        def ubc_dma(eng, c0, c1):
            ub_sl = ub[c0 * F:c1 * F]
            ub_b = bass.AP(tensor=ub_sl.tensor, offset=ub_sl.offset,
                           ap=[[0, 128]] + [list(a) for a in ub_sl.ap])
            eng.dma_start(out=ubc_sb[:, c0 * F:c1 * F], in_=ub_b)

        # W trigger groups: two singles first (fast pipeline start), then
        # pairs; issued interleaved with half-size x groups so class 0's
        # main GEMM is never waiting on x.
        WGROUPS = [[0], [1], [2, 3], [4, 5], [6, 7], [8, 9], [10, 11], [12]]
        wt_of = {}

        def wt_dma(gi):
            cls = WGROUPS[gi]
            wt = wtp.tile([128, len(cls), KCH, F], f16, tag="wt",
                          name=f"wtg{gi}")
            nc.sync.dma_start(
                out=wt, in_=d["wt"][:, cls[0] * KF:(cls[-1] + 1) * KF
                                    ].rearrange("p (c k f) -> p c k f",
                                                k=KCH, f=F))
            for i, c in enumerate(cls):
                wt_of[c] = (wt, i)

        def wtc(c):
            t, i = wt_of[c]
            return t[:, i, :, :]

        # Sync queue: wt0 (class 0's G gates on it), the small misc/m16/
        # early-u tiles, then the last two x groups (consumed last by
        # class 0's GEMM) and the remaining W groups which trail the
        # compute easily.  ScalarE queue in parallel: the first six x
        # groups in consumption order, then the remaining u classes.
        XG = 2  # x chunks per staging DMA
        NXG = KCH // XG
        xt_tiles = [None] * NXG
        wt_dma(0)

        def xt_dma(g, eng):
            xg = singles.tile([128, XG, B], f16, tag=f"xt{g}",
                              name=f"xt{g}")
            eng.dma_start(
                out=xg, in_=d["xt"][:, g * XG * B:(g + 1) * XG * B
                                    ].rearrange("p (k b) -> p k b", b=B))
            xt_tiles[g] = xg

        for g in range(NXG - 2):
            xt_dma(g, nc.scalar)
        nc.sync.dma_start(out=misc_sb, in_=d["misc"][:, 0:MW])
        nc.sync.dma_start(out=m16_sb, in_=d["m16"][:, 0:1])
        ubc_dma(nc.sync, 0, 4)
        for g in range(NXG - 2, NXG):
            xt_dma(g, nc.sync)
        ubc_dma(nc.scalar, 4, CL)
        for gi in range(1, len(WGROUPS)):
            wt_dma(gi)

        negm_sb = singles.tile([F, CL], f32, tag="negm")
        nc.vector.tensor_sub(negm_sb, misc_sb[:, :CL], misc_sb[:, CL:2 * CL])

        def xchunk(k):
            return xt_tiles[k // XG][:, k % XG, :]

        st = [dict() for _ in range(CL)]

        def emit_sigma(cr):
            """r = G u (DVE STT), fused broadcast dots (PE), 1/sigma chain."""
            s = st[cr]
            scr = smp.tile([128, F], f32, tag="scr")
            rcol = misc_sb[:, RUC + 2 * cr:RUC + 2 * cr + 1]
            nc.vector.scalar_tensor_tensor(
                out=scr, in0=s["G"], scalar=1.0,
                in1=ubc_sb[:, cr * F:(cr + 1) * F],
                op0=mult, op1=mult, accum_out=rcol)
            # fused dots+broadcast: lhsT = r replicated 128x (stride-0
            # free dim), rhs = [r | u] -> out[m, :] = [r.r, u.r] for all m
            db = dbp.tile([128, 2], f32, tag="db")
            s["db"] = db
            rrep = bass.AP(tensor=rcol.tensor, offset=rcol.offset,
                           ap=[list(rcol.ap[0]), [0, 128]])
            nc.tensor.matmul(
                db, lhsT=rrep,
                rhs=misc_sb[:, RUC + 2 * cr:RUC + 2 * cr + 2],
                start=True, stop=True)

        def emit_chain(cr):
            s = st[cr]
            db = s["db"]
            recip = smp.tile([128, 1], f32, tag="recip")
            nc.vector.reciprocal(recip, db[:, 0:1])
            invs2 = smp.tile([128, 1], f32, tag="invs2")
            nc.vector.tensor_mul(invs2, recip, db[:, 1:2])
            lnr = smp.tile([128, 1], f32, tag="lnr")
            nc.scalar.activation(out=lnr, in_=invs2, func=AF.Ln)
            invs0 = smp.tile([128, 1], f32, tag="invs0")
            nc.scalar.activation(out=invs0, in_=lnr, func=AF.Exp, scale=0.5)
            # one Newton step y1 = (y0 + a/y0)/2 tightens the LUT
            # exp(0.5 ln a) sqrt estimate to ~1 ulp; probs error is
            # a large multiple of the relative sigma error.
            ry = smp.tile([128, 1], f32, tag="ry")
            nc.vector.reciprocal(ry, invs0)
            ar = smp.tile([128, 1], f32, tag="ar")
            nc.vector.tensor_mul(ar, invs2, ry)
            hsum = smp.tile([128, 1], f32, tag="hsum")
            nc.vector.tensor_add(hsum, invs0, ar)
            invs = smp.tile([128, 1], f32, tag="invs")
            nc.vector.tensor_scalar_mul(invs, hsum, 0.5)
            s["invs"] = invs

        for it in range(CL + 1):
            cb, cd = it, it - 1
            last = cb == CL - 1

            # ------ Scalar first: Square(cd) so dist2 can follow main(it)
            if 0 <= cd < CL:
                s = st[cd]
                sq = sqp.tile([F, B], f16, tag="sq")
                s["sq"] = sq
                nc.scalar.activation(
                    out=sq, in_=s["zT"], func=AF.Square,
                    bias=negm_sb[:, cd:cd + 1], scale=s["invs"])

            # ------ PE: G(cb) then main GEMM(cb), same wt chunks
            if cb < CL:
                s = st[cb]
                wt = wtc(cb)
                G = gps.tile([128, F], f32, tag="G")
                s["G"] = G
                for k in range(KCH):
                    nc.tensor.matmul(
                        G, lhsT=wt[:, k, :], rhs=wt[:, k, :],
                        start=(k == 0), stop=(k == KCH - 1))
                # last class: sigma path ahead of the main GEMM so its
                # chain finishes during the GEMM instead of in the drain
                if last:
                    emit_sigma(cb)
                zT = zps.tile([F, B], f32, tag="zT")
                s["zT"] = zT
                for k in range(KCH):
                    nc.tensor.matmul(
                        zT, lhsT=wt[:, k, :], rhs=xchunk(k),
                        start=(k == 0), stop=(k == KCH - 1))

            # ------ D(cd): dist2 + probs (dist2 emitted after main(it)
            # so the PE never stalls waiting on Square)
            if 0 <= cd < CL:
                s = st[cd]
                d2 = dps.tile([1, B], f32, tag="d2")
                nc.tensor.matmul(d2, lhsT=ones_sb, rhs=s["sq"],
                                 start=True, stop=True)
                probs_c = smp.tile([1, B], f32, tag="probs_c")
                nc.scalar.activation(
                    out=probs_c, in_=d2, func=AF.Exp, scale=-0.5)
                nc.sync.dma_start(out=d["out"][0:1, cd * B:(cd + 1) * B],
                                  in_=probs_c)

            # ------ sigma(cb): r, dots, chain
            if cb < CL:
                if not last:
                    emit_sigma(cb)
                emit_chain(cb)


def _build():
    nc = _Bacc(trn_type="TRN2", target_bir_lowering=False, debug=False,
               num_devices=NCORES)
    f32 = mybir.dt.float32
    f16 = mybir.dt.float16
    d = {
        "wt": nc.dram_tensor("wt", [128, CL * KCH * F], f16,
                             kind="ExternalInput").ap(),
        "xt": nc.dram_tensor("xt", [128, KCH * B], f16,
                             kind="ExternalInput").ap(),
        "misc": nc.dram_tensor("misc", [128, MW], f32,
                               kind="ExternalInput").ap(),
        "m16": nc.dram_tensor("m16", [128, 1], f16,
                              kind="ExternalInput").ap(),
        "ubflat": nc.dram_tensor("ubflat", [CL * F], f16,
                                 kind="ExternalInput").ap(),
        "out": nc.dram_tensor("out", [1, CL * B], f32,
                              kind="ExternalOutput").ap(),
    }
    with tile.TileContext(nc) as tc:
        _emit(tc, d)
    nc.compile()
    return nc


def _get_nc():
    global _NC
    if _NC is None:
        _NC = _build()
    return _NC


def make_in_maps(inputs):
    x = np.ascontiguousarray(inputs["x"], dtype=np.float32)
    W = np.ascontiguousarray(inputs["W"], dtype=np.float32)
    b = np.ascontiguousarray(inputs["b"], dtype=np.float32)
    u = np.ascontiguousarray(inputs["u"], dtype=np.float32)
    c = np.ascontiguousarray(inputs["c"], dtype=np.float32)
    pad = CPAD - C
    Wp = np.concatenate([W, W[:pad]], axis=0)
    bp = np.concatenate([b, b[:pad]], axis=0)
    up = np.concatenate([u, u[:pad]], axis=0)
    cp = np.concatenate([c, c[:pad]], axis=0)
    # pre-permute to per-partition-contiguous fp16 layouts so device DMAs
    # are simple 2D copies (cheap SP triggers, full-row HBM reads):
    # wt[p, c, k, f] = W[c, f, 128k+p];  xt[p, k, b] = x[b, 128k+p]
    WT = Wp.transpose(0, 2, 1).reshape(CPAD, KCH, 128, F)
    xt = np.ascontiguousarray(x.T.reshape(KCH, 128, B).transpose(1, 0, 2)
                              .reshape(128, KCH * B).astype(np.float16))
    m16 = np.ones((128, 1), dtype=np.float16)
    in_maps = []
    for ci in range(NCORES):
        sl = slice(ci * CL, (ci + 1) * CL)
        ruc = np.zeros((128, 2 * CL), dtype=np.float32)
        ruc[:, 1::2] = up[sl].T
        in_maps.append({
            "wt": np.ascontiguousarray(
                WT[sl].transpose(2, 0, 1, 3).reshape(128, CL * KCH * F)
                .astype(np.float16)),
            "xt": xt,
            "ubflat": np.ascontiguousarray(
                up[sl].reshape(-1).astype(np.float16)),
            "misc": np.ascontiguousarray(np.concatenate(
                [bp[sl].T, cp[sl].T, ruc], axis=1)),
            "m16": m16,
        })
    return in_maps


def run_spmd(in_maps, **kw):
    from concourse.bass_utils import run_bass_kernel_spmd
    return run_bass_kernel_spmd(_get_nc(), in_maps, list(range(NCORES)), **kw)


def gather_output(results):
    rows = np.concatenate(
        [results[i]["out"].reshape(CL, B) for i in range(NCORES)], axis=0)
    return np.ascontiguousarray(rows[:C].T)  # [B, C] float32


def kernel(**inputs):
    bkr = run_spmd(make_in_maps(inputs))
    return gather_output(bkr.results)


# revision 13
# speedup vs baseline: 1.1396x; 1.0250x over previous
"""Trainium2 Bass kernel for nn_MultiLinearCentroids (vq_codebook).

Reference math per class c (C=100, F=128, E=2048, B=512):
  one spectral-norm power-iteration step:
    sigma_c = || W_c (W_c^T u_c) || / || W_c^T u_c ||
  z = x @ W_c^T / sigma_c + b_c                         [B, F]
  probs[:, c] = exp(-||c_c - z||^2 / 2)                 [B]

Sharding: class dim padded 100 -> 104 = 8 cores x 13 classes. x replicated.
Host does only layout transforms (transpose / slice / concat / dtype cast);
all math (including sigma) runs on device.

Key design points (vs. the 127us pipeline that computed t = W^T u as a
GpSimd/DVE elementwise multiply + segmented reduce):
  - sigma via the Gram matrix: G_c = W_c W_c^T is 16 PE matmuls per class
    on the SAME wt chunks the main GEMM uses (lhsT = rhs = wt[:,k,:],
    fp32 PSUM accumulate, ~57ns each).  Then r = G u is ONE DVE STT
    (in0 = G from PSUM, in1 = u broadcast, accum_out fp32 written into
    the column slot right before u's column).
  - fused dots+broadcast: one fp32 matmul with lhsT = the r column
    replicated 128x via a stride-0 free dim and rhs = the [r | u] column
    pair gives [rr, ru] = [r.r, u.r] (u.r == ||W^T u||^2) broadcast onto
    all 128 partitions in one shot -- no 1-partition dots, no ScalarE
    copy, no ones-matmul broadcast.
  - per-class 1/sigma chain on the broadcast pair: exp(0.5 ln(ru/rr)) +
    one Newton step (Ln/Exp/Square all live in the
    natural_log_exp_and_others ACT table set -> single table load).
  - sq = Square(zT * invs + (b - c)) one ScalarE op -> fp16; dist2 =
    ones^T @ sq (fp16 PE partition reduce); probs row = Exp(-0.5 dist2),
    DMA'd out per class.
  - W, x ship as FP16 (PE 1 cyc/row, HBM traffic ~9.5MB); host
    pre-permutes W/x to per-partition-contiguous layouts so each DMA is
    a plain 2D copy.  x ships in 8 half-groups interleaved with the W
    class groups so class 0 is never DMA-blocked.
  - The PE p-states ramp 0.65 -> 1.2 -> 2.4 GHz over ~3us of continuous
    work, so a handful of dependency-free warmup matmuls on an
    uninitialized SBUF tile run during the DMA prologue to finish the
    ramp before class 0's real matmuls issue.
  - Pipeline per iteration it: G(it) -> main GEMM(it) -> dist2(it-1) ->
    fused dots(it) on the PE queue; r-STT(it) + chain(it) on DVE;
    Square(it-1)/probs(it-1) + chain Ln/Exp on ScalarE.  GpSimd idle.
    PE is the critical engine at ~4.9us/class.  The last class hoists
    its dots ahead of its main GEMM to shorten the drain.
"""

import numpy as np

import concourse.bass as bass
import concourse.tile as tile
from concourse import bacc


class _Bacc(bacc.Bacc):
    """Bacc whose ACT-table pass only sees natural_log_exp_and_others.

    The default pass picks the first table set containing each function
    (natural_log for Ln, exp_and_others for Exp), which alternates sets
    every class = many table loads x ~2.7us. Ln, Exp and Square all live in
    natural_log_exp_and_others, so one load covers the whole kernel."""

    def insert_act_table_loads(self):
        from concourse.hw_specs import get_activation_tables
        has_activation = any(
            isinstance(i, bacc.mybir.InstActivation)
            for b in self.main_func.blocks
            for i in b.instructions
        )
        if not has_activation:
            return
        tables = [(k, v if k == "natural_log_exp_and_others" else type(v)())
                  for k, v in get_activation_tables(self.m.arch).items()]
        bacc._bass_rust.insert_act_table_loads(self, tables)


from concourse import mybir

B = 512
C = 100
E = 2048
F = 128
NCORES = 8
CPAD = 104
CL = CPAD // NCORES  # 13 classes per core
KCH = E // 128       # 16 contraction chunks
KF = KCH * F
NWARM = 5            # p-state warmup matmuls during the DMA prologue

# misc column layout: [b.T | c.T]; (r, u) column pairs live in the
# separate fp16 ruc tile so the dots matmul runs as cheap fp16
MW = 2 * CL

_NC = None


def _emit(tc, d):
    nc = tc.nc
    f32 = mybir.dt.float32
    f16 = mybir.dt.float16
    mult = mybir.AluOpType.mult
    AF = mybir.ActivationFunctionType

    import contextlib
    ctx = contextlib.ExitStack()
    with ctx:
        singles = ctx.enter_context(tc.tile_pool(name="singles", bufs=1))
        wtp = ctx.enter_context(tc.tile_pool(name="wtp", bufs=8))
        sqp = ctx.enter_context(tc.tile_pool(name="sqp", bufs=2))
        smp = ctx.enter_context(tc.tile_pool(name="smp", bufs=4))
        zps = ctx.enter_context(tc.tile_pool(name="zps", bufs=2, space="PSUM"))
        gps = ctx.enter_context(tc.tile_pool(name="gps", bufs=2, space="PSUM"))
        dps = ctx.enter_context(tc.tile_pool(name="dps", bufs=1, space="PSUM"))
        dbp = ctx.enter_context(tc.tile_pool(name="dbp", bufs=2, space="PSUM"))
        wup = ctx.enter_context(tc.tile_pool(name="wup", bufs=1, space="PSUM"))

        # --- PE p-state warmup: dependency-free matmuls on an
        # uninitialized SBUF tile; results land in a dedicated junk PSUM
        # bank that is never read.
        warm_sb = singles.tile([128, B], f16, tag="warm")
        nc.gpsimd.memset(warm_sb, 0.0)
        warm_ps = wup.tile([128, B], f32, tag="warmp")
        for _ in range(NWARM):
            nc.tensor.matmul(warm_ps, lhsT=warm_sb[:, 0:128], rhs=warm_sb,
                             start=True, stop=True)

        # --- input staging across the two hardware DGE queues (SP +
        # ScalarE): the DMA engines ramp like the PE (~170 GB/s early),
        # so the critical prologue bytes (wt0 + x + u for early classes)
        # are split so both queues land them in parallel.
        ub = d["ubflat"]
        ubc_sb = singles.tile([128, CL * F], f16, tag="ubc")
        misc_sb = singles.tile([128, MW], f32, tag="misc")
        ruc_sb = singles.tile([128, 2 * CL], f16, tag="ruc")
        m16_sb = singles.tile([128, 1], f16, tag="m16")
        ones_sb = m16_sb[:, 0:1]

        def ubc_dma(eng, c0, c1):
            ub_sl = ub[c0 * F:c1 * F]
            ub_b = bass.AP(tensor=ub_sl.tensor, offset=ub_sl.offset,
                           ap=[[0, 128]] + [list(a) for a in ub_sl.ap])
            eng.dma_start(out=ubc_sb[:, c0 * F:c1 * F], in_=ub_b)

        # W trigger groups: two singles first (fast pipeline start), then
        # pairs; issued interleaved with half-size x groups so class 0's
        # main GEMM is never waiting on x.
        WGROUPS = [[0], [1], [2, 3], [4, 5], [6, 7], [8, 9], [10, 11], [12]]
        wt_of = {}

        def wt_dma(gi):
            cls = WGROUPS[gi]
            wt = wtp.tile([128, len(cls), KCH, F], f16, tag="wt",
                          name=f"wtg{gi}")
            nc.sync.dma_start(
                out=wt, in_=d["wt"][:, cls[0] * KF:(cls[-1] + 1) * KF
                                    ].rearrange("p (c k f) -> p c k f",
                                                k=KCH, f=F))
            for i, c in enumerate(cls):
                wt_of[c] = (wt, i)

        def wtc(c):
            t, i = wt_of[c]
            return t[:, i, :, :]

        # Sync queue: wt0 (class 0's G gates on it), the small misc/m16/
        # early-u tiles, then the last two x groups (consumed last by
        # class 0's GEMM) and the remaining W groups which trail the
        # compute easily.  ScalarE queue in parallel: the first six x
        # groups in consumption order, then the remaining u classes.
        XG = 2  # x chunks per staging DMA
        NXG = KCH // XG
        xt_tiles = [None] * NXG
        wt_dma(0)

        def xt_dma(g, eng):
            xg = singles.tile([128, XG, B], f16, tag=f"xt{g}",
                              name=f"xt{g}")
            eng.dma_start(
                out=xg, in_=d["xt"][:, g * XG * B:(g + 1) * XG * B
                                    ].rearrange("p (k b) -> p k b", b=B))
            xt_tiles[g] = xg

        for g in range(NXG - 2):
            xt_dma(g, nc.scalar)
        nc.sync.dma_start(out=misc_sb, in_=d["misc"][:, 0:MW])
        nc.sync.dma_start(out=ruc_sb, in_=d["ruc16"][:, 0:2 * CL])
        nc.sync.dma_start(out=m16_sb, in_=d["m16"][:, 0:1])
        ubc_dma(nc.sync, 0, 4)
        for g in range(NXG - 2, NXG):
            xt_dma(g, nc.sync)
        ubc_dma(nc.scalar, 4, CL)
        for gi in range(1, len(WGROUPS)):
            wt_dma(gi)

        negm_sb = singles.tile([F, CL], f32, tag="negm")
        nc.vector.tensor_sub(negm_sb, misc_sb[:, :CL], misc_sb[:, CL:2 * CL])

        def xchunk(k):
            return xt_tiles[k // XG][:, k % XG, :]

        st = [dict() for _ in range(CL)]

        def emit_sigma(cr):
            """r = G u (DVE STT), fused broadcast dots (PE), 1/sigma chain."""
            s = st[cr]
            scr = smp.tile([128, F], f32, tag="scr")
            rcol = ruc_sb[:, 2 * cr:2 * cr + 1]
            with nc.allow_low_precision(reason="r feeds fp16 dots matmul"):
                nc.vector.scalar_tensor_tensor(
                    out=scr, in0=s["G"], scalar=1.0,
                    in1=ubc_sb[:, cr * F:(cr + 1) * F],
                    op0=mult, op1=mult, accum_out=rcol)
            # fused dots+broadcast: lhsT = r replicated 128x (stride-0
            # free dim), rhs = [r | u] -> out[m, :] = [r.r, u.r] for all m
            db = dbp.tile([128, 2], f32, tag="db")
            s["db"] = db
            rrep = bass.AP(tensor=rcol.tensor, offset=rcol.offset,
                           ap=[list(rcol.ap[0]), [0, 128]])
            nc.tensor.matmul(
                db, lhsT=rrep,
                rhs=ruc_sb[:, 2 * cr:2 * cr + 2],
                start=True, stop=True)

        def emit_chain(cr):
            s = st[cr]
            db = s["db"]
            recip = smp.tile([128, 1], f32, tag="recip")
            nc.vector.reciprocal(recip, db[:, 0:1])
            invs2 = smp.tile([128, 1], f32, tag="invs2")
            nc.vector.tensor_mul(invs2, recip, db[:, 1:2])
            lnr = smp.tile([128, 1], f32, tag="lnr")
            nc.scalar.activation(out=lnr, in_=invs2, func=AF.Ln)
            invs0 = smp.tile([128, 1], f32, tag="invs0")
            nc.scalar.activation(out=invs0, in_=lnr, func=AF.Exp, scale=0.5)
            # one Newton step y1 = (y0 + a/y0)/2 tightens the LUT
            # exp(0.5 ln a) sqrt estimate to ~1 ulp; probs error is
            # a large multiple of the relative sigma error.
            ry = smp.tile([128, 1], f32, tag="ry")
            nc.vector.reciprocal(ry, invs0)
            ar = smp.tile([128, 1], f32, tag="ar")
            nc.vector.tensor_mul(ar, invs2, ry)
            hsum = smp.tile([128, 1], f32, tag="hsum")
            nc.vector.tensor_add(hsum, invs0, ar)
            invs = smp.tile([128, 1], f32, tag="invs")
            nc.vector.tensor_scalar_mul(invs, hsum, 0.5)
            s["invs"] = invs

        for it in range(CL + 1):
            cb, cd = it, it - 1
            last = cb == CL - 1

            # ------ Scalar first: Square(cd) so dist2 can follow main(it)
            if 0 <= cd < CL:
                s = st[cd]
                sq = sqp.tile([F, B], f16, tag="sq")
                s["sq"] = sq
                nc.scalar.activation(
                    out=sq, in_=s["zT"], func=AF.Square,
                    bias=negm_sb[:, cd:cd + 1], scale=s["invs"])

            # ------ PE: G(cb) then main GEMM(cb), same wt chunks
            if cb < CL:
                s = st[cb]
                wt = wtc(cb)
                G = gps.tile([128, F], f32, tag="G")
                s["G"] = G
                for k in range(KCH):
                    nc.tensor.matmul(
                        G, lhsT=wt[:, k, :], rhs=wt[:, k, :],
                        start=(k == 0), stop=(k == KCH - 1))
                # last class: sigma path ahead of the main GEMM so its
                # chain finishes during the GEMM instead of in the drain
                if last:
                    emit_sigma(cb)
                zT = zps.tile([F, B], f32, tag="zT")
                s["zT"] = zT
                for k in range(KCH):
                    nc.tensor.matmul(
                        zT, lhsT=wt[:, k, :], rhs=xchunk(k),
                        start=(k == 0), stop=(k == KCH - 1))

            # ------ D(cd): dist2 + probs (dist2 emitted after main(it)
            # so the PE never stalls waiting on Square)
            if 0 <= cd < CL:
                s = st[cd]
                d2 = dps.tile([1, B], f32, tag="d2")
                nc.tensor.matmul(d2, lhsT=ones_sb, rhs=s["sq"],
                                 start=True, stop=True)
                probs_c = smp.tile([1, B], f32, tag="probs_c")
                nc.scalar.activation(
                    out=probs_c, in_=d2, func=AF.Exp, scale=-0.5)
                nc.sync.dma_start(out=d["out"][0:1, cd * B:(cd + 1) * B],
                                  in_=probs_c)

            # ------ sigma(cb): r, dots, chain
            if cb < CL:
                if not last:
                    emit_sigma(cb)
                emit_chain(cb)


def _build():
    nc = _Bacc(trn_type="TRN2", target_bir_lowering=False, debug=False,
               num_devices=NCORES)
    f32 = mybir.dt.float32
    f16 = mybir.dt.float16
    d = {
        "wt": nc.dram_tensor("wt", [128, CL * KCH * F], f16,
                             kind="ExternalInput").ap(),
        "xt": nc.dram_tensor("xt", [128, KCH * B], f16,
                             kind="ExternalInput").ap(),
        "misc": nc.dram_tensor("misc", [128, MW], f32,
                               kind="ExternalInput").ap(),
        "ruc16": nc.dram_tensor("ruc16", [128, 2 * CL], f16,
                                kind="ExternalInput").ap(),
        "m16": nc.dram_tensor("m16", [128, 1], f16,
                              kind="ExternalInput").ap(),
        "ubflat": nc.dram_tensor("ubflat", [CL * F], f16,
                                 kind="ExternalInput").ap(),
        "out": nc.dram_tensor("out", [1, CL * B], f32,
                              kind="ExternalOutput").ap(),
    }
    with tile.TileContext(nc) as tc:
        _emit(tc, d)
    nc.compile()
    return nc


def _get_nc():
    global _NC
    if _NC is None:
        _NC = _build()
    return _NC


def make_in_maps(inputs):
    x = np.ascontiguousarray(inputs["x"], dtype=np.float32)
    W = np.ascontiguousarray(inputs["W"], dtype=np.float32)
    b = np.ascontiguousarray(inputs["b"], dtype=np.float32)
    u = np.ascontiguousarray(inputs["u"], dtype=np.float32)
    c = np.ascontiguousarray(inputs["c"], dtype=np.float32)
    pad = CPAD - C
    Wp = np.concatenate([W, W[:pad]], axis=0)
    bp = np.concatenate([b, b[:pad]], axis=0)
    up = np.concatenate([u, u[:pad]], axis=0)
    cp = np.concatenate([c, c[:pad]], axis=0)
    # pre-permute to per-partition-contiguous fp16 layouts so device DMAs
    # are simple 2D copies (cheap SP triggers, full-row HBM reads):
    # wt[p, c, k, f] = W[c, f, 128k+p];  xt[p, k, b] = x[b, 128k+p]
    WT = Wp.transpose(0, 2, 1).reshape(CPAD, KCH, 128, F)
    xt = np.ascontiguousarray(x.T.reshape(KCH, 128, B).transpose(1, 0, 2)
                              .reshape(128, KCH * B).astype(np.float16))
    m16 = np.ones((128, 1), dtype=np.float16)
    in_maps = []
    for ci in range(NCORES):
        sl = slice(ci * CL, (ci + 1) * CL)
        ruc = np.zeros((128, 2 * CL), dtype=np.float16)
        ruc[:, 1::2] = up[sl].T.astype(np.float16)
        in_maps.append({
            "wt": np.ascontiguousarray(
                WT[sl].transpose(2, 0, 1, 3).reshape(128, CL * KCH * F)
                .astype(np.float16)),
            "xt": xt,
            "ubflat": np.ascontiguousarray(
                up[sl].reshape(-1).astype(np.float16)),
            "misc": np.ascontiguousarray(np.concatenate(
                [bp[sl].T, cp[sl].T], axis=1)),
            "ruc16": np.ascontiguousarray(ruc),
            "m16": m16,
        })
    return in_maps


def run_spmd(in_maps, **kw):
    from concourse.bass_utils import run_bass_kernel_spmd
    return run_bass_kernel_spmd(_get_nc(), in_maps, list(range(NCORES)), **kw)


def gather_output(results):
    rows = np.concatenate(
        [results[i]["out"].reshape(CL, B) for i in range(NCORES)], axis=0)
    return np.ascontiguousarray(rows[:C].T)  # [B, C] float32


def kernel(**inputs):
    bkr = run_spmd(make_in_maps(inputs))
    return gather_output(bkr.results)


# revision 14
# speedup vs baseline: 1.1509x; 1.0099x over previous
"""Trainium2 Bass kernel for nn_MultiLinearCentroids (vq_codebook).

Reference math per class c (C=100, F=128, E=2048, B=512):
  one spectral-norm power-iteration step:
    sigma_c = || W_c (W_c^T u_c) || / || W_c^T u_c ||
  z = x @ W_c^T / sigma_c + b_c                         [B, F]
  probs[:, c] = exp(-||c_c - z||^2 / 2)                 [B]

Sharding: class dim padded 100 -> 104 = 8 cores x 13 classes. x replicated.
Host does only layout transforms (transpose / slice / concat / dtype cast);
all math (including sigma) runs on device.

Key design points (vs. the 127us pipeline that computed t = W^T u as a
GpSimd/DVE elementwise multiply + segmented reduce):
  - sigma via the Gram matrix: G_c = W_c W_c^T is 16 PE matmuls per class
    on the SAME wt chunks the main GEMM uses (lhsT = rhs = wt[:,k,:],
    fp32 PSUM accumulate, ~57ns each).  Then r = G u is ONE DVE STT
    (in0 = G from PSUM, in1 = u broadcast, accum_out fp32 written into
    the column slot right before u's column).
  - fused dots+broadcast: one fp32 matmul with lhsT = the r column
    replicated 128x via a stride-0 free dim and rhs = the [r | u] column
    pair gives [rr, ru] = [r.r, u.r] (u.r == ||W^T u||^2) broadcast onto
    all 128 partitions in one shot -- no 1-partition dots, no ScalarE
    copy, no ones-matmul broadcast.
  - per-class 1/sigma chain on the broadcast pair: exp(0.5 ln(ru/rr)) +
    one Newton step (Ln/Exp/Square all live in the
    natural_log_exp_and_others ACT table set -> single table load).
  - sq = Square(zT * invs + (b - c)) one ScalarE op -> fp16; dist2 =
    ones^T @ sq (fp16 PE partition reduce); probs row = Exp(-0.5 dist2),
    DMA'd out per class.
  - W, x ship as FP16 (PE 1 cyc/row, HBM traffic ~9.5MB); host
    pre-permutes W/x to per-partition-contiguous layouts so each DMA is
    a plain 2D copy.  x ships in 8 half-groups interleaved with the W
    class groups so class 0 is never DMA-blocked.
  - The PE p-states ramp 0.65 -> 1.2 -> 2.4 GHz over ~3us of continuous
    work, so a handful of dependency-free warmup matmuls on an
    uninitialized SBUF tile run during the DMA prologue to finish the
    ramp before class 0's real matmuls issue.
  - Pipeline per iteration it: G(it) -> main GEMM(it) -> dist2(it-1) ->
    fused dots(it) on the PE queue; r-STT(it) + chain(it) on DVE;
    Square(it-1)/probs(it-1) + chain Ln/Exp on ScalarE.  GpSimd idle.
    PE is the critical engine at ~4.9us/class.  The last class hoists
    its dots ahead of its main GEMM to shorten the drain.
"""

import numpy as np

import concourse.bass as bass
import concourse.tile as tile
from concourse import bacc


class _Bacc(bacc.Bacc):
    """Bacc whose ACT-table pass only sees natural_log_exp_and_others.

    The default pass picks the first table set containing each function
    (natural_log for Ln, exp_and_others for Exp), which alternates sets
    every class = many table loads x ~2.7us. Ln, Exp and Square all live in
    natural_log_exp_and_others, so one load covers the whole kernel."""

    def insert_act_table_loads(self):
        from concourse.hw_specs import get_activation_tables
        has_activation = any(
            isinstance(i, bacc.mybir.InstActivation)
            for b in self.main_func.blocks
            for i in b.instructions
        )
        if not has_activation:
            return
        tables = [(k, v if k == "natural_log_exp_and_others" else type(v)())
                  for k, v in get_activation_tables(self.m.arch).items()]
        bacc._bass_rust.insert_act_table_loads(self, tables)


from concourse import mybir

B = 512
C = 100
E = 2048
F = 128
NCORES = 8
CPAD = 104
CL = CPAD // NCORES  # 13 classes per core
KCH = E // 128       # 16 contraction chunks
KF = KCH * F
NWARM = 9            # p-state warmup matmuls during the DMA prologue

# misc column layout: [b.T | c.T]; (r, u) column pairs live in the
# separate fp16 ruc tile so the dots matmul runs as cheap fp16
MW = 2 * CL

_NC = None


def _emit(tc, d):
    nc = tc.nc
    f32 = mybir.dt.float32
    f16 = mybir.dt.float16
    mult = mybir.AluOpType.mult
    AF = mybir.ActivationFunctionType

    import contextlib
    ctx = contextlib.ExitStack()
    with ctx:
        singles = ctx.enter_context(tc.tile_pool(name="singles", bufs=1))
        wtp = ctx.enter_context(tc.tile_pool(name="wtp", bufs=8))
        sqp = ctx.enter_context(tc.tile_pool(name="sqp", bufs=2))
        smp = ctx.enter_context(tc.tile_pool(name="smp", bufs=4))
        zps = ctx.enter_context(tc.tile_pool(name="zps", bufs=2, space="PSUM"))
        gps = ctx.enter_context(tc.tile_pool(name="gps", bufs=2, space="PSUM"))
        dps = ctx.enter_context(tc.tile_pool(name="dps", bufs=1, space="PSUM"))
        dbp = ctx.enter_context(tc.tile_pool(name="dbp", bufs=2, space="PSUM"))
        wup = ctx.enter_context(tc.tile_pool(name="wup", bufs=1, space="PSUM"))

        # --- PE p-state warmup: dependency-free matmuls on an
        # uninitialized SBUF tile; results land in a dedicated junk PSUM
        # bank that is never read.
        warm_sb = singles.tile([128, B], f16, tag="warm")
        nc.gpsimd.memset(warm_sb, 0.0)
        warm_ps = wup.tile([128, B], f32, tag="warmp")
        for _ in range(NWARM):
            nc.tensor.matmul(warm_ps, lhsT=warm_sb[:, 0:128], rhs=warm_sb,
                             start=True, stop=True)

        # --- input staging across the two hardware DGE queues (SP +
        # ScalarE): the DMA engines ramp like the PE (~170 GB/s early),
        # so the critical prologue bytes (wt0 + x + u for early classes)
        # are split so both queues land them in parallel.
        ub = d["ubflat"]
        ubc_sb = singles.tile([128, CL * F], f16, tag="ubc")
        misc_sb = singles.tile([128, MW], f32, tag="misc")
        ruc_sb = singles.tile([128, 2 * CL], f16, tag="ruc")
        m16_sb = singles.tile([128, 1], f16, tag="m16")
        ones_sb = m16_sb[:, 0:1]

        def ubc_dma(eng, c0, c1):
            ub_sl = ub[c0 * F:c1 * F]
            ub_b = bass.AP(tensor=ub_sl.tensor, offset=ub_sl.offset,
                           ap=[[0, 128]] + [list(a) for a in ub_sl.ap])
            eng.dma_start(out=ubc_sb[:, c0 * F:c1 * F], in_=ub_b)

        # W trigger groups: two singles first (fast pipeline start), then
        # pairs; issued interleaved with half-size x groups so class 0's
        # main GEMM is never waiting on x.
        WGROUPS = [[0], [1], [2, 3], [4, 5], [6, 7], [8, 9], [10, 11], [12]]
        wt_of = {}

        def wt_dma(gi):
            cls = WGROUPS[gi]
            wt = wtp.tile([128, len(cls), KCH, F], f16, tag="wt",
                          name=f"wtg{gi}")
            nc.sync.dma_start(
                out=wt, in_=d["wt"][:, cls[0] * KF:(cls[-1] + 1) * KF
                                    ].rearrange("p (c k f) -> p c k f",
                                                k=KCH, f=F))
            for i, c in enumerate(cls):
                wt_of[c] = (wt, i)

        def wtc(c):
            t, i = wt_of[c]
            return t[:, i, :, :]

        # Sync queue: wt0 (class 0's G gates on it), the small misc/m16/
        # early-u tiles, then the last two x groups (consumed last by
        # class 0's GEMM) and the remaining W groups which trail the
        # compute easily.  ScalarE queue in parallel: the first six x
        # groups in consumption order, then the remaining u classes.
        XG = 2  # x chunks per staging DMA
        NXG = KCH // XG
        xt_tiles = [None] * NXG
        wt_dma(0)

        def xt_dma(g, eng):
            xg = singles.tile([128, XG, B], f16, tag=f"xt{g}",
                              name=f"xt{g}")
            eng.dma_start(
                out=xg, in_=d["xt"][:, g * XG * B:(g + 1) * XG * B
                                    ].rearrange("p (k b) -> p k b", b=B))
            xt_tiles[g] = xg

        for g in range(NXG - 2):
            xt_dma(g, nc.scalar)
        nc.sync.dma_start(out=misc_sb, in_=d["misc"][:, 0:MW])
        nc.sync.dma_start(out=ruc_sb, in_=d["ruc16"][:, 0:2 * CL])
        nc.sync.dma_start(out=m16_sb, in_=d["m16"][:, 0:1])
        ubc_dma(nc.sync, 0, 4)
        for g in range(NXG - 2, NXG):
            xt_dma(g, nc.sync)
        ubc_dma(nc.scalar, 4, CL)
        for gi in range(1, len(WGROUPS)):
            wt_dma(gi)

        negm_sb = singles.tile([F, CL], f32, tag="negm")
        nc.vector.tensor_sub(negm_sb, misc_sb[:, :CL], misc_sb[:, CL:2 * CL])

        def xchunk(k):
            return xt_tiles[k // XG][:, k % XG, :]

        st = [dict() for _ in range(CL)]

        def emit_sigma(cr):
            """r = G u (DVE STT), fused broadcast dots (PE), 1/sigma chain."""
            s = st[cr]
            scr = smp.tile([128, F], f32, tag="scr")
            rcol = ruc_sb[:, 2 * cr:2 * cr + 1]
            with nc.allow_low_precision(reason="r feeds fp16 dots matmul"):
                nc.vector.scalar_tensor_tensor(
                    out=scr, in0=s["G"], scalar=1.0,
                    in1=ubc_sb[:, cr * F:(cr + 1) * F],
                    op0=mult, op1=mult, accum_out=rcol)
            # fused dots+broadcast: lhsT = r replicated 128x (stride-0
            # free dim), rhs = [r | u] -> out[m, :] = [r.r, u.r] for all m
            db = dbp.tile([128, 2], f32, tag="db")
            s["db"] = db
            rrep = bass.AP(tensor=rcol.tensor, offset=rcol.offset,
                           ap=[list(rcol.ap[0]), [0, 128]])
            nc.tensor.matmul(
                db, lhsT=rrep,
                rhs=ruc_sb[:, 2 * cr:2 * cr + 2],
                start=True, stop=True)

        def emit_chain(cr):
            s = st[cr]
            db = s["db"]
            recip = smp.tile([128, 1], f32, tag="recip")
            nc.vector.reciprocal(recip, db[:, 0:1])
            invs2 = smp.tile([128, 1], f32, tag="invs2")
            nc.vector.tensor_mul(invs2, recip, db[:, 1:2])
            lnr = smp.tile([128, 1], f32, tag="lnr")
            nc.scalar.activation(out=lnr, in_=invs2, func=AF.Ln)
            invs0 = smp.tile([128, 1], f32, tag="invs0")
            nc.scalar.activation(out=invs0, in_=lnr, func=AF.Exp, scale=0.5)
            # one Newton step y1 = (y0 + a/y0)/2 tightens the LUT
            # exp(0.5 ln a) sqrt estimate to ~1 ulp; probs error is
            # a large multiple of the relative sigma error.
            ry = smp.tile([128, 1], f32, tag="ry")
            nc.vector.reciprocal(ry, invs0)
            ar = smp.tile([128, 1], f32, tag="ar")
            nc.vector.tensor_mul(ar, invs2, ry)
            hsum = smp.tile([128, 1], f32, tag="hsum")
            nc.vector.tensor_add(hsum, invs0, ar)
            invs = smp.tile([128, 1], f32, tag="invs")
            nc.vector.tensor_scalar_mul(invs, hsum, 0.5)
            s["invs"] = invs

        for it in range(CL + 1):
            cb, cd = it, it - 1
            last = cb == CL - 1

            # ------ Scalar first: Square(cd) so dist2 can follow main(it)
            if 0 <= cd < CL:
                s = st[cd]
                sq = sqp.tile([F, B], f16, tag="sq")
                s["sq"] = sq
                nc.scalar.activation(
                    out=sq, in_=s["zT"], func=AF.Square,
                    bias=negm_sb[:, cd:cd + 1], scale=s["invs"])

            # ------ PE: G(cb) then main GEMM(cb), same wt chunks
            if cb < CL:
                s = st[cb]
                wt = wtc(cb)
                G = gps.tile([128, F], f32, tag="G")
                s["G"] = G
                for k in range(KCH):
                    nc.tensor.matmul(
                        G, lhsT=wt[:, k, :], rhs=wt[:, k, :],
                        start=(k == 0), stop=(k == KCH - 1))
                # last class: sigma path ahead of the main GEMM so its
                # chain finishes during the GEMM instead of in the drain
                if last:
                    emit_sigma(cb)
                zT = zps.tile([F, B], f32, tag="zT")
                s["zT"] = zT
                for k in range(KCH):
                    nc.tensor.matmul(
                        zT, lhsT=wt[:, k, :], rhs=xchunk(k),
                        start=(k == 0), stop=(k == KCH - 1))

            # ------ D(cd): dist2 + probs (dist2 emitted after main(it)
            # so the PE never stalls waiting on Square)
            if 0 <= cd < CL:
                s = st[cd]
                d2 = dps.tile([1, B], f32, tag="d2")
                nc.tensor.matmul(d2, lhsT=ones_sb, rhs=s["sq"],
                                 start=True, stop=True)
                probs_c = smp.tile([1, B], f32, tag="probs_c")
                nc.scalar.activation(
                    out=probs_c, in_=d2, func=AF.Exp, scale=-0.5)
                nc.sync.dma_start(out=d["out"][0:1, cd * B:(cd + 1) * B],
                                  in_=probs_c)

            # ------ sigma(cb): r, dots, chain
            if cb < CL:
                if not last:
                    emit_sigma(cb)
                emit_chain(cb)


def _build():
    nc = _Bacc(trn_type="TRN2", target_bir_lowering=False, debug=False,
               num_devices=NCORES)
    f32 = mybir.dt.float32
    f16 = mybir.dt.float16
    d = {
        "wt": nc.dram_tensor("wt", [128, CL * KCH * F], f16,
                             kind="ExternalInput").ap(),
        "xt": nc.dram_tensor("xt", [128, KCH * B], f16,
                             kind="ExternalInput").ap(),
        "misc": nc.dram_tensor("misc", [128, MW], f32,
                               kind="ExternalInput").ap(),
        "ruc16": nc.dram_tensor("ruc16", [128, 2 * CL], f16,
                                kind="ExternalInput").ap(),
        "m16": nc.dram_tensor("m16", [128, 1], f16,
                              kind="ExternalInput").ap(),
        "ubflat": nc.dram_tensor("ubflat", [CL * F], f16,
                                 kind="ExternalInput").ap(),
        "out": nc.dram_tensor("out", [1, CL * B], f32,
                              kind="ExternalOutput").ap(),
    }
    with tile.TileContext(nc) as tc:
        _emit(tc, d)
    nc.compile()
    return nc


def _get_nc():
    global _NC
    if _NC is None:
        _NC = _build()
    return _NC


def make_in_maps(inputs):
    x = np.ascontiguousarray(inputs["x"], dtype=np.float32)
    W = np.ascontiguousarray(inputs["W"], dtype=np.float32)
    b = np.ascontiguousarray(inputs["b"], dtype=np.float32)
    u = np.ascontiguousarray(inputs["u"], dtype=np.float32)
    c = np.ascontiguousarray(inputs["c"], dtype=np.float32)
    pad = CPAD - C
    Wp = np.concatenate([W, W[:pad]], axis=0)
    bp = np.concatenate([b, b[:pad]], axis=0)
    up = np.concatenate([u, u[:pad]], axis=0)
    cp = np.concatenate([c, c[:pad]], axis=0)
    # pre-permute to per-partition-contiguous fp16 layouts so device DMAs
    # are simple 2D copies (cheap SP triggers, full-row HBM reads):
    # wt[p, c, k, f] = W[c, f, 128k+p];  xt[p, k, b] = x[b, 128k+p]
    WT = Wp.transpose(0, 2, 1).reshape(CPAD, KCH, 128, F)
    xt = np.ascontiguousarray(x.T.reshape(KCH, 128, B).transpose(1, 0, 2)
                              .reshape(128, KCH * B).astype(np.float16))
    m16 = np.ones((128, 1), dtype=np.float16)
    in_maps = []
    for ci in range(NCORES):
        sl = slice(ci * CL, (ci + 1) * CL)
        ruc = np.zeros((128, 2 * CL), dtype=np.float16)
        ruc[:, 1::2] = up[sl].T.astype(np.float16)
        in_maps.append({
            "wt": np.ascontiguousarray(
                WT[sl].transpose(2, 0, 1, 3).reshape(128, CL * KCH * F)
                .astype(np.float16)),
            "xt": xt,
            "ubflat": np.ascontiguousarray(
                up[sl].reshape(-1).astype(np.float16)),
            "misc": np.ascontiguousarray(np.concatenate(
                [bp[sl].T, cp[sl].T], axis=1)),
            "ruc16": np.ascontiguousarray(ruc),
            "m16": m16,
        })
    return in_maps


def run_spmd(in_maps, **kw):
    from concourse.bass_utils import run_bass_kernel_spmd
    return run_bass_kernel_spmd(_get_nc(), in_maps, list(range(NCORES)), **kw)


def gather_output(results):
    rows = np.concatenate(
        [results[i]["out"].reshape(CL, B) for i in range(NCORES)], axis=0)
    return np.ascontiguousarray(rows[:C].T)  # [B, C] float32


def kernel(**inputs):
    bkr = run_spmd(make_in_maps(inputs))
    return gather_output(bkr.results)
